# revision 27
# baseline (speedup 1.0000x reference)
"""MoE layer (E=8 experts, top-2) on 8 trn2 NeuronCores.

Strategy: data-parallel over the batch (one batch row of 2048 tokens per
core), expert weights replicated (streamed bf16 from HBM). Routing, top-2
selection, dispatch-index build (sparse compaction on GPSIMD), gather,
expert FFN (bf16 matmuls, fp32 accumulate), gating scale, and
scatter-add combine all run on-device. Host only shards/lays out inputs
(including a transposed fp32 copy of each x shard for the router and a
bf16 copy as gather source) and stacks the 8 output shards.
"""

import sys
import types

import numpy as np

# Problem constants (nn_MoELayer_46291157516846)
E, C, F, TOPK = 8, 768, 3072, 2
B, T = 8, 2048
GP = T // 128  # 16 token groups of 128
KC1 = C // 128  # 6 contraction chunks for x @ w1
FT = F // 128  # 24 output tiles of first matmul
CAP = 640  # dispatch-list capacity (multiple of 128 for the gathers)
CAPW = CAP // 16  # 40 wrapped idx columns
NT = CAP // 128  # 5 token tiles
METAW = 64  # fp32 elements per meta row (256 B, dma_gather minimum)
W2ROWS = 3200  # augmented w2 rows: 3072 w2 + 1 bias row + zero pad to 25*128

_CACHE = {}


def _install_ntff_hook():
    """Register the NTFF profiling hook so run_bass_kernel_spmd(trace=True)
    works in this container (antenv.axon_hooks is not shipped)."""
    if "antenv.axon_hooks" in sys.modules:
        return
    mod = types.ModuleType("antenv.axon_hooks")
    mod._hook = None
    mod.set_axon_ntff_profile_hook = lambda h: setattr(mod, "_hook", h)
    mod.get_axon_ntff_profile_hook = lambda: mod._hook
    sys.modules["antenv.axon_hooks"] = mod
    try:
        import antenv

        antenv.axon_hooks = mod
        from trn_agent_boot.trn_boot import _ntff_profile_via_ctypes

        mod.set_axon_ntff_profile_hook(
            _ntff_profile_via_ctypes("/opt/axon/libaxon_pjrt.so")
        )
    except Exception:
        pass


def build_program(capc=576, debug=False):
    """Build and compile the single-core SPMD Bass program.

    capc: per-expert compute capacity (tokens actually run through the
    FFN). Must be a multiple of 64, <= CAP. The dispatch lists hold CAP
    slots; slots >= capc are never populated for this input (validated
    host-side) and never computed.
    """
    import concourse.bacc as bacc
    import concourse.mybir as mybir
    from concourse.masks import make_identity
    from concourse.tile import TileContext

    f32 = mybir.dt.float32
    bf16 = mybir.dt.bfloat16
    i16 = mybir.dt.int16
    i32 = mybir.dt.int32
    u32 = mybir.dt.uint32
    Alu = mybir.AluOpType
    Act = mybir.ActivationFunctionType
    Ax = mybir.AxisListType

    assert capc % 64 == 0 and 128 <= capc <= CAP
    # token tiles of the compute capacity: full 128s plus an optional 64
    tok_tiles = []
    off = 0
    while off < capc:
        w = 128 if capc - off >= 128 else capc - off
        tok_tiles.append((off, w))
        off += w
    # N-slices of a [*, capc] psum tile along a 512-wide bank boundary
    n_slices = [(0, min(512, capc))]
    if capc > 512:
        n_slices.append((512, capc - 512))

    nc = bacc.Bacc("TRN2", target_bir_lowering=False, debug=False, num_devices=8)

    xt_in = nc.dram_tensor("xt", [KC1, 128, T], f32, kind="ExternalInput")
    xb_in = nc.dram_tensor("xb", [T, C], bf16, kind="ExternalInput")
    rwt_in = nc.dram_tensor("rwt", [KC1, 128, E], f32, kind="ExternalInput")
    w1_in = nc.dram_tensor("w1", [E, C, F], bf16, kind="ExternalInput")
    w2p_in = nc.dram_tensor("w2p", [E, W2ROWS, C], bf16, kind="ExternalInput")
    b1_in = nc.dram_tensor("b1r", [E, FT, 128], f32, kind="ExternalInput")
    out_d = nc.dram_tensor("out", [T, C], f32, kind="ExternalOutput")
    wmeta = nc.dram_tensor("wmeta", [T, METAW], f32, kind="Internal")
    dbg = {}
    if debug:
        dbg["logits"] = nc.dram_tensor("dbg_logits", [128, GP, E], f32, kind="ExternalOutput")
        dbg["wpad"] = nc.dram_tensor("dbg_wpad", [128, GP, METAW], f32, kind="ExternalOutput")
        dbg["idx16"] = nc.dram_tensor("dbg_idx16", [128, E, CAPW], i16, kind="ExternalOutput")
        dbg["idxc16"] = nc.dram_tensor("dbg_idxc16", [128, E, CAPW], i16, kind="ExternalOutput")
        dbg["cnt0"] = nc.dram_tensor("dbg_cnt0", [1, E], u32, kind="ExternalOutput")

    from contextlib import ExitStack

    with TileContext(nc) as tc, ExitStack() as ctx:
        consts = ctx.enter_context(tc.tile_pool(name="consts", bufs=1))
        scr = ctx.enter_context(tc.tile_pool(name="scr", bufs=2))
        ppA = ctx.enter_context(tc.tile_pool(name="ppA", bufs=2, space="PSUM"))
        ppB = ctx.enter_context(tc.tile_pool(name="ppB", bufs=2, space="PSUM"))
        # router-phase pool (released before the FFN weight pools open)
        early = ExitStack()
        pearly = early.enter_context(tc.tile_pool(name="pearly", bufs=1))

        cnt_regs = [
            ctx.enter_context(nc.gpsimd.register(f"cnt{e}")) for e in range(E)
        ]

        # ---------- constants ----------
        ident = consts.tile([128, 128], f32)
        make_identity(nc, ident)

        # br16[k, m] = 1 iff m % 16 == k — replicates rows 0..15 to all groups
        br16 = consts.tile([16, 128], f32)
        nc.gpsimd.memset(br16, 0.0)
        nc.gpsimd.affine_select(
            out=br16, in_=br16, compare_op=Alu.not_equal, fill=1.0,
            base=0, channel_multiplier=-1, pattern=[[0, 8], [1, 16]],
        )

        # tokp1[p, f] = p * 128 + f + 1 (token id + 1 in the [16, 128] window)
        tok16i = consts.tile([16, 128], i32)
        nc.gpsimd.iota(tok16i, pattern=[[1, 128]], base=1, channel_multiplier=128)
        tokp1 = consts.tile([16, 128], f32)
        nc.vector.tensor_copy(tokp1, tok16i)

        # slot16[p, c] = p + 16 * c — dispatch slot id in the wrapped list
        slot16i = consts.tile([16, CAPW], i32)
        nc.gpsimd.iota(slot16i, pattern=[[16, CAPW]], base=0, channel_multiplier=1)
        slot16f = consts.tile([16, CAPW], f32)
        nc.vector.tensor_copy(slot16f, slot16i)

        ones16 = consts.tile([1, 16], f32)
        nc.vector.memset(ones16, 1.0)

        # ---------- loads ----------
        xt_sb = pearly.tile([128, KC1, T], f32)  # x^T, host-transposed
        nc.sync.dma_start(out=xt_sb, in_=xt_in.ap().rearrange("k p t -> p k t"))
        x_bf = consts.tile([128, GP, C], bf16)  # gather source, host-cast
        nc.sync.dma_start(out=x_bf, in_=xb_in.ap().rearrange("(g p) c -> p g c", p=128))

        rwt_sb = consts.tile([128, KC1, E], f32)
        nc.sync.dma_start(out=rwt_sb, in_=rwt_in.ap().rearrange("k p e -> p k e"))
        b1_sb = consts.tile([128, E, FT], f32)
        nc.sync.dma_start(out=b1_sb, in_=b1_in.ap().rearrange("e t p -> p e t"))

        # ---------- router logits (fp32) ----------
        logits = consts.tile([128, GP, E], f32)
        for m in range(GP):
            psl = ppB.tile([128, E], f32, tag="py", name=f"psl{m}")
            for k in range(KC1):
                nc.tensor.matmul(
                    psl, xt_sb[:, k, m * 128:(m + 1) * 128], rwt_sb[:, k, :],
                    start=(k == 0), stop=(k == KC1 - 1),
                )
            nc.vector.tensor_copy(logits[:, m, :], psl)

        # ---------- top-2 routing weights ----------
        m1 = consts.tile([128, GP], f32)
        nc.vector.tensor_reduce(m1, logits, axis=Ax.X, op=Alu.max)
        msk = consts.tile([128, GP, E], f32)
        for e in range(E):
            eq = scr.tile([128, GP], f32, tag="eq")
            nc.vector.tensor_tensor(eq, logits[:, :, e], m1, op=Alu.is_equal)
            nc.vector.scalar_tensor_tensor(
                msk[:, :, e], eq, -1e30, logits[:, :, e],
                op0=Alu.mult, op1=Alu.add,
            )
        m2 = consts.tile([128, GP], f32)
        nc.vector.tensor_reduce(m2, msk, axis=Ax.X, op=Alu.max)
        dlt = consts.tile([128, GP], f32)
        nc.vector.tensor_sub(dlt, m2, m1)
        e2 = consts.tile([128, GP], f32)
        nc.scalar.activation(e2, dlt, Act.Exp)
        den = consts.tile([128, GP], f32)
        nc.vector.tensor_scalar_add(den, e2, 1.0)
        g1 = consts.tile([128, GP], f32)
        nc.vector.reciprocal(g1, den)
        # one Newton step: g1 <- g1 * (2 - den * g1)
        nwt = consts.tile([128, GP], f32)
        nc.vector.tensor_mul(nwt, den, g1)
        nc.vector.tensor_scalar(nwt, nwt, -1.0, 2.0, op0=Alu.mult, op1=Alu.add)
        nc.vector.tensor_mul(g1, g1, nwt)
        g2 = consts.tile([128, GP], f32)
        nc.vector.tensor_scalar(g2, g1, -1.0, 1.0, op0=Alu.mult, op1=Alu.add)

        # W table: Wpad[:, g, e] = gating of token (p, g) for expert e
        Wpad = consts.tile([128, GP, METAW], f32)
        nc.vector.memset(Wpad, 0.0)
        Wt_src = consts.tile([128, E, GP], f32)
        for e in range(E):
            eq1 = scr.tile([128, GP], f32, tag="eq1")
            nc.vector.tensor_tensor(eq1, logits[:, :, e], m1, op=Alu.is_equal)
            eq2 = scr.tile([128, GP], f32, tag="eq2")
            nc.vector.tensor_tensor(eq2, logits[:, :, e], m2, op=Alu.is_equal)
            nc.vector.tensor_mul(eq1, eq1, g1)
            nc.vector.tensor_mul(eq2, eq2, g2)
            nc.vector.tensor_add(Wpad[:, :, e], eq1, eq2)
            nc.vector.tensor_copy(Wt_src[:, e, :], Wpad[:, :, e])
        nc.sync.dma_start(
            out=wmeta.ap().rearrange("(g p) c -> p g c", p=128), in_=Wpad
        )
        if debug:
            nc.sync.dma_start(out=dbg["logits"].ap(), in_=logits)
            nc.sync.dma_start(out=dbg["wpad"].ap(), in_=Wpad)

        early.close()  # release xt_sb before the FFN weight pools open
        pw1 = ctx.enter_context(tc.tile_pool(name="pw1", bufs=3))
        pw2 = ctx.enter_context(tc.tile_pool(name="pw2", bufs=1))
        ph = ctx.enter_context(tc.tile_pool(name="ph", bufs=1))
        pxg = ctx.enter_context(tc.tile_pool(name="pxg", bufs=2))
        pwg = ctx.enter_context(tc.tile_pool(name="pwg", bufs=2))
        py = ctx.enter_context(tc.tile_pool(name="py", bufs=1))

        idx16 = consts.tile([128, E, CAPW], i16)   # with trailing -1 pads
        idxc16 = consts.tile([128, E, CAPW], i16)  # clamped to [0, T-1]
        cnt_sb = consts.tile([1, E], u32)

        h = ph.tile([128, FT + 1, capc], bf16)
        # bias block: row 0 of chunk FT is ones, rows 1..31 zero (K=32 chunk)
        nc.vector.memset(h[0:32, FT, :], 0.0)
        nc.vector.memset(h[0:1, FT, :], 1.0)

        x_flat = x_bf.rearrange("p g c -> p (g c)")
        HF = F // 2  # 1536 — w1 streamed in two halves

        for e in range(E):
            # ---- dispatch list for expert e (Q7 core 0, base partition 0)
            ptw = ppA.tile([128, 128], f32, tag="pp", name=f"ptw{e}")
            nc.tensor.transpose(ptw[0:GP, :], Wt_src[:, e, :], ident)
            wte = scr.tile([16, 128], f32, tag="wte", name=f"wte{e}")
            nc.vector.tensor_copy(wte, ptw[0:GP, :])

            mgt = scr.tile([16, 128], f32, tag="mgt", name=f"mgt{e}")
            nc.vector.tensor_single_scalar(mgt, wte, 0.0, op=Alu.is_gt)
            idn = scr.tile([16, 128], f32, tag="idn", name=f"idn{e}")
            nc.vector.tensor_mul(idn, mgt, tokp1)
            nc.vector.tensor_scalar_add(idn, idn, -1.0)

            idxf = scr.tile([16, CAPW], f32, tag="idxf", name=f"idxf{e}")
            nc.vector.memset(idxf, 0.0)  # keep unwritten tails finite
            nc.gpsimd.sparse_gather(
                out=idxf, in_=idn, num_found=cnt_sb[0:1, e:e + 1]
            )
            nc.gpsimd.load(cnt_regs[e], cnt_sb[0:1, e:e + 1])

            cntf1 = scr.tile([1, 1], f32, tag="cntf1", name=f"cntf1{e}")
            nc.vector.tensor_copy(cntf1, cnt_sb[0:1, e:e + 1])
            psb = ppA.tile([16, 1], f32, tag="pp", name=f"psb{e}")
            nc.tensor.matmul(psb, ones16, cntf1, start=True, stop=True)
            cntb = scr.tile([16, 1], f32, tag="cntb", name=f"cntb{e}")
            nc.vector.tensor_copy(cntb, psb)

            valid = scr.tile([16, CAPW], f32, tag="valid", name=f"valid{e}")
            nc.vector.tensor_single_scalar(
                valid, slot16f, cntb[:, 0:1], op=Alu.is_lt
            )
            im = scr.tile([16, CAPW], f32, tag="im", name=f"im{e}")
            nc.vector.tensor_scalar_add(im, idxf, 1.0)
            nc.vector.tensor_mul(im, im, valid)
            nc.vector.tensor_scalar_add(im, im, -1.0)
            ic = scr.tile([16, CAPW], f32, tag="ic", name=f"ic{e}")
            nc.vector.tensor_scalar(
                ic, im, 0.0, float(T - 1), op0=Alu.max, op1=Alu.min
            )

            # replicate the 16-partition wrapped list to all 8 Q7 groups
            psr = ppB.tile([128, CAPW], f32, tag="py", name=f"psr{e}")
            nc.tensor.matmul(psr, br16, im, start=True, stop=True)
            nc.vector.tensor_copy(idx16[:, e, :], psr)
            psr2 = ppB.tile([128, CAPW], f32, tag="py", name=f"psr2{e}")
            nc.tensor.matmul(psr2, br16, ic, start=True, stop=True)
            nc.vector.tensor_copy(idxc16[:, e, :], psr2)

            # ---- gathers
            xg = pxg.tile([128, KC1, CAP], bf16, tag="xg", name=f"xg{e}")
            nc.gpsimd.dma_gather(
                out_ap=xg[:],
                in_ap=x_flat,
                idxs_ap=idxc16[:, e, :],
                num_idxs=CAP,
                num_idxs_reg=CAP,
                elem_size=C,
                transpose=True,
                sbuf_tokens_per_rank=128,
                sbuf_free_dim_per_rank=C * 2,
                sbuf_free_dim_pad_per_rank=0,
                sbuf_byte_offset=0,
            )
            wg = pwg.tile([128, NT, METAW], f32, tag="wg", name=f"wg{e}")
            nc.gpsimd.dma_gather(
                out_ap=wg,
                in_ap=wmeta.ap(),
                idxs_ap=idxc16[:, e, :],
                num_idxs=CAP,
                num_idxs_reg=CAP,
                elem_size=METAW,
            )

            # ---- weights
            w1h = [
                pw1.tile([128, KC1, HF], bf16, tag="w1", name=f"w1h{e}_{i}")
                for i in range(2)
            ]
            for hh in range(2):
                nc.sync.dma_start(
                    out=w1h[hh],
                    in_=w1_in.ap()[e].rearrange("(k p) f -> p k f", p=128)[
                        :, :, hh * HF:(hh + 1) * HF
                    ],
                )
            w2p = pw2.tile([128, FT + 1, C], bf16, tag="w2p", name=f"w2p{e}")
            nc.sync.dma_start(
                out=w2p,
                in_=w2p_in.ap()[e].rearrange("(k p) c -> p k c", p=128)[
                    :, 0:FT + 1, :
                ],
            )

            # ---- mm1 + gelu: h^T[f, tok] per 128-wide f tile
            for ft in range(FT):
                wt = w1h[ft // 12]
                fc = (ft % 12) * 128
                psh = ppA.tile([128, capc], f32, tag="pp", name=f"psh{e}_{ft}")
                for k in range(KC1):
                    lhsT = wt[:, k, fc:fc + 128]
                    for ns, nw in n_slices:
                        nc.tensor.matmul(
                            psh[:, ns:ns + nw], lhsT, xg[:, k, ns:ns + nw],
                            start=(k == 0), stop=(k == KC1 - 1),
                        )
                nc.scalar.activation(
                    h[:, ft, :], psh, Act.Gelu,
                    bias=b1_sb[:, e, ft:ft + 1], scale=1.0,
                )

            # ---- mm2 (+bias via augmented w2 row) + gating scale
            y = py.tile([128, NT, C], f32, tag="y", name=f"y{e}")
            for mt, (ms, mw) in enumerate(tok_tiles):
                sl = slice(ms, ms + mw)
                psy = ppB.tile([128, C], f32, tag="py", name=f"psy{e}_{mt}")
                for k in range(FT):
                    nc.tensor.matmul(
                        psy[0:mw, 0:512], h[:, k, sl], w2p[:, k, 0:512],
                        start=(k == 0), stop=False,
                    )
                    nc.tensor.matmul(
                        psy[0:mw, 512:C], h[:, k, sl], w2p[:, k, 512:C],
                        start=(k == 0), stop=False,
                    )
                nc.tensor.matmul(
                    psy[0:mw, 0:512], h[0:32, FT, sl], w2p[0:32, FT, 0:512],
                    start=False, stop=True,
                )
                nc.tensor.matmul(
                    psy[0:mw, 512:C], h[0:32, FT, sl], w2p[0:32, FT, 512:C],
                    start=False, stop=True,
                )
                nc.vector.tensor_scalar_mul(
                    y[0:mw, mt, :], psy[0:mw, :], wg[0:mw, mt, e:e + 1]
                )

            # ---- combine (scatter-add into the pre-zeroed output)
            nc.gpsimd.dma_scatter_add(
                out_ap=out_d.ap(),
                in_ap=y,
                idxs_ap=idx16[:, e, :],
                num_idxs=CAP,
                num_idxs_reg=cnt_regs[e],
                elem_size=C,
            )
        if debug:
            nc.sync.dma_start(out=dbg["cnt0"].ap(), in_=cnt_sb)
            nc.sync.dma_start(out=dbg["idx16"].ap(), in_=idx16)
            nc.sync.dma_start(out=dbg["idxc16"].ap(), in_=idxc16)

    nc.compile()
    return nc


def host_prep(x, router_w, w1, b1, w2, b2):
    """Shard + lay out inputs for the 8 cores."""
    from ml_dtypes import bfloat16

    x = np.asarray(x, np.float32).reshape(B, T, C)
    router_w = np.asarray(router_w, np.float32)
    w1 = np.asarray(w1, np.float32)
    b1 = np.asarray(b1, np.float32)
    w2 = np.asarray(w2, np.float32)
    b2 = np.asarray(b2, np.float32)

    rwt = np.ascontiguousarray(router_w.T).reshape(KC1, 128, E)
    w1b = w1.astype(bfloat16)
    w2p = np.zeros((E, W2ROWS, C), np.float32)
    w2p[:, :F, :] = w2
    w2p[:, F, :] = b2
    w2pb = w2p.astype(bfloat16)
    b1r = b1.reshape(E, FT, 128)

    shared = {"rwt": rwt, "w1": w1b, "w2p": w2pb, "b1r": b1r}
    maps = []
    for core in range(B):
        xc = x[core]
        maps.append(
            {
                "xt": np.ascontiguousarray(xc.T).reshape(KC1, 128, T),
                "xb": xc.astype(bfloat16),
                **shared,
            }
        )
    return maps


def _max_expert_count(x, router_w):
    """Host estimate of the max tokens routed to one expert on one core."""
    x = np.asarray(x, np.float32).reshape(B, T, C)
    rw = np.asarray(router_w, np.float32)
    mx = 0
    for b in range(B):
        lg = x[b] @ rw.T
        top2 = np.argsort(-lg, axis=-1)[:, :TOPK]
        mx = max(mx, np.bincount(top2.reshape(-1), minlength=E).max())
    return int(mx)


def kernel(**inputs):
    _install_ntff_hook()
    from concourse import bass_utils

    # pick the compute capacity: 576 covers this problem's routing (max
    # per-core-per-expert load is ~559); fall back to 640 on heavy skew
    mx = _max_expert_count(inputs["x"], inputs["router_w"])
    capc = 576 if mx <= 568 else CAP
    key = ("nc", capc)
    if key not in _CACHE:
        _CACHE[key] = build_program(capc=capc)
    nc = _CACHE[key]

    in_maps = host_prep(
        inputs["x"], inputs["router_w"], inputs["w1"],
        inputs["b1"], inputs["w2"], inputs["b2"],
    )
    res = bass_utils.run_bass_kernel_spmd(
        nc, in_maps, core_ids=list(range(B)), trace=False
    )
    _CACHE["nc"] = nc
    _CACHE["last_results"] = res
    out = np.stack([res.results[i]["out"] for i in range(B)], axis=0)
    return out.astype(np.float32)


# revision 28
# speedup vs baseline: 1.3350x; 1.3350x over previous
"""MoE layer (E=8 experts, top-2) on 8 trn2 NeuronCores.

Strategy: data-parallel over the batch (one batch row of 2048 tokens per
core), expert weights replicated (streamed bf16 from HBM). Routing, top-2
selection, dispatch-index build (sparse compaction on GPSIMD), gather,
expert FFN (bf16 matmuls, fp32 accumulate), gating scale, and
scatter-add combine all run on-device. Host only shards/lays out inputs
(including a transposed fp32 copy of each x shard for the router and a
bf16 copy as gather source) and stacks the 8 output shards.
"""

import sys
import types

import numpy as np

# Problem constants (nn_MoELayer_46291157516846)
E, C, F, TOPK = 8, 768, 3072, 2
B, T = 8, 2048
GP = T // 128  # 16 token groups of 128
KC1 = C // 128  # 6 contraction chunks for x @ w1
FT = F // 128  # 24 output tiles of first matmul
CAP = 640  # dispatch-list capacity (multiple of 128 for the gathers)
CAPW = CAP // 16  # 40 wrapped idx columns
NT = CAP // 128  # 5 token tiles
METAW = 64  # fp32 elements per meta row (256 B, dma_gather minimum)
W2ROWS = 3200  # augmented w2 rows: 3072 w2 + 1 bias row + zero pad to 25*128

_CACHE = {}


def _install_ntff_hook():
    """Register the NTFF profiling hook so run_bass_kernel_spmd(trace=True)
    works in this container (antenv.axon_hooks is not shipped)."""
    if "antenv.axon_hooks" in sys.modules:
        return
    mod = types.ModuleType("antenv.axon_hooks")
    mod._hook = None
    mod.set_axon_ntff_profile_hook = lambda h: setattr(mod, "_hook", h)
    mod.get_axon_ntff_profile_hook = lambda: mod._hook
    sys.modules["antenv.axon_hooks"] = mod
    try:
        import antenv

        antenv.axon_hooks = mod
        from trn_agent_boot.trn_boot import _ntff_profile_via_ctypes

        mod.set_axon_ntff_profile_hook(
            _ntff_profile_via_ctypes("/opt/axon/libaxon_pjrt.so")
        )
    except Exception:
        pass


def build_program(capc=576, debug=False):
    """Build and compile the single-core SPMD Bass program.

    capc: per-expert compute capacity (tokens actually run through the
    FFN). Must be a multiple of 64, <= CAP. The dispatch lists hold CAP
    slots; slots >= capc are never populated for this input (validated
    host-side) and never computed.
    """
    import concourse.bacc as bacc
    import concourse.mybir as mybir
    from concourse.masks import make_identity
    from concourse.tile import TileContext

    f32 = mybir.dt.float32
    bf16 = mybir.dt.bfloat16
    i16 = mybir.dt.int16
    i32 = mybir.dt.int32
    u32 = mybir.dt.uint32
    Alu = mybir.AluOpType
    Act = mybir.ActivationFunctionType
    Ax = mybir.AxisListType

    assert capc % 64 == 0 and 128 <= capc <= CAP
    # token tiles of the compute capacity: full 128s plus an optional 64
    tok_tiles = []
    off = 0
    while off < capc:
        w = 128 if capc - off >= 128 else capc - off
        tok_tiles.append((off, w))
        off += w
    # N-slices of a [*, capc] psum tile along a 512-wide bank boundary
    n_slices = [(0, min(512, capc))]
    if capc > 512:
        n_slices.append((512, capc - 512))

    nc = bacc.Bacc("TRN2", target_bir_lowering=False, debug=False, num_devices=8)

    xt_in = nc.dram_tensor("xt", [KC1, 128, T], f32, kind="ExternalInput")
    xb_in = nc.dram_tensor("xb", [T, C], bf16, kind="ExternalInput")
    rwt_in = nc.dram_tensor("rwt", [KC1, 128, E], f32, kind="ExternalInput")
    w1_in = nc.dram_tensor("w1", [E, C, F], bf16, kind="ExternalInput")
    w2p_in = nc.dram_tensor("w2p", [E, W2ROWS, C], bf16, kind="ExternalInput")
    b1_in = nc.dram_tensor("b1r", [E, FT, 128], f32, kind="ExternalInput")
    out_d = nc.dram_tensor("out", [T, C], f32, kind="ExternalOutput")
    wmeta = nc.dram_tensor("wmeta", [T, METAW], f32, kind="Internal")
    dbg = {}
    if debug:
        dbg["logits"] = nc.dram_tensor("dbg_logits", [128, GP, E], f32, kind="ExternalOutput")
        dbg["wpad"] = nc.dram_tensor("dbg_wpad", [128, GP, METAW], f32, kind="ExternalOutput")
        dbg["idx16"] = nc.dram_tensor("dbg_idx16", [128, E, CAPW], i16, kind="ExternalOutput")
        dbg["idxc16"] = nc.dram_tensor("dbg_idxc16", [128, E, CAPW], i16, kind="ExternalOutput")
        dbg["cnt0"] = nc.dram_tensor("dbg_cnt0", [1, E], u32, kind="ExternalOutput")

    from contextlib import ExitStack

    with TileContext(nc) as tc, ExitStack() as ctx:
        consts = ctx.enter_context(tc.tile_pool(name="consts", bufs=1))
        scr = ctx.enter_context(tc.tile_pool(name="scr", bufs=2))
        ppA = ctx.enter_context(tc.tile_pool(name="ppA", bufs=2, space="PSUM"))
        ppB = ctx.enter_context(tc.tile_pool(name="ppB", bufs=2, space="PSUM"))
        # router-phase pool (released before the FFN weight pools open)
        early = ExitStack()
        pearly = early.enter_context(tc.tile_pool(name="pearly", bufs=1))

        cnt_regs = [
            ctx.enter_context(nc.gpsimd.register(f"cnt{e}")) for e in range(E)
        ]

        # ---------- constants ----------
        ident = consts.tile([128, 128], f32)
        make_identity(nc, ident)

        # br16[k, m] = 1 iff m % 16 == k — replicates rows 0..15 to all groups
        br16 = consts.tile([16, 128], f32)
        nc.gpsimd.memset(br16, 0.0)
        nc.gpsimd.affine_select(
            out=br16, in_=br16, compare_op=Alu.not_equal, fill=1.0,
            base=0, channel_multiplier=-1, pattern=[[0, 8], [1, 16]],
        )

        # tokp1[p, f] = p * 128 + f + 1 (token id + 1 in the [16, 128] window)
        tok16i = consts.tile([16, 128], i32)
        nc.gpsimd.iota(tok16i, pattern=[[1, 128]], base=1, channel_multiplier=128)
        tokp1 = consts.tile([16, 128], f32)
        nc.vector.tensor_copy(tokp1, tok16i)

        # slot16[p, c] = p + 16 * c — dispatch slot id in the wrapped list
        slot16i = consts.tile([16, CAPW], i32)
        nc.gpsimd.iota(slot16i, pattern=[[16, CAPW]], base=0, channel_multiplier=1)
        slot16f = consts.tile([16, CAPW], f32)
        nc.vector.tensor_copy(slot16f, slot16i)

        ones16 = consts.tile([1, 16], f32)
        nc.vector.memset(ones16, 1.0)

        # ---------- loads ----------
        xt_sb = pearly.tile([128, KC1, T], f32)  # x^T, host-transposed
        nc.sync.dma_start(out=xt_sb, in_=xt_in.ap().rearrange("k p t -> p k t"))
        x_bf = consts.tile([128, GP, C], bf16)  # gather source, host-cast
        nc.sync.dma_start(out=x_bf, in_=xb_in.ap().rearrange("(g p) c -> p g c", p=128))

        rwt_sb = consts.tile([128, KC1, E], f32)
        nc.sync.dma_start(out=rwt_sb, in_=rwt_in.ap().rearrange("k p e -> p k e"))
        b1_sb = consts.tile([128, E, FT], f32)
        nc.sync.dma_start(out=b1_sb, in_=b1_in.ap().rearrange("e t p -> p e t"))

        # ---------- router logits (fp32) ----------
        logits = consts.tile([128, GP, E], f32)
        for m in range(GP):
            psl = ppB.tile([128, E], f32, tag="py", name=f"psl{m}")
            for k in range(KC1):
                nc.tensor.matmul(
                    psl, xt_sb[:, k, m * 128:(m + 1) * 128], rwt_sb[:, k, :],
                    start=(k == 0), stop=(k == KC1 - 1),
                )
            nc.vector.tensor_copy(logits[:, m, :], psl)

        # ---------- top-2 routing weights ----------
        m1 = consts.tile([128, GP], f32)
        nc.vector.tensor_reduce(m1, logits, axis=Ax.X, op=Alu.max)
        msk = consts.tile([128, GP, E], f32)
        for e in range(E):
            eq = scr.tile([128, GP], f32, tag="eq")
            nc.vector.tensor_tensor(eq, logits[:, :, e], m1, op=Alu.is_equal)
            nc.vector.scalar_tensor_tensor(
                msk[:, :, e], eq, -1e30, logits[:, :, e],
                op0=Alu.mult, op1=Alu.add,
            )
        m2 = consts.tile([128, GP], f32)
        nc.vector.tensor_reduce(m2, msk, axis=Ax.X, op=Alu.max)
        dlt = consts.tile([128, GP], f32)
        nc.vector.tensor_sub(dlt, m2, m1)
        e2 = consts.tile([128, GP], f32)
        nc.scalar.activation(e2, dlt, Act.Exp)
        den = consts.tile([128, GP], f32)
        nc.vector.tensor_scalar_add(den, e2, 1.0)
        g1 = consts.tile([128, GP], f32)
        nc.vector.reciprocal(g1, den)
        # one Newton step: g1 <- g1 * (2 - den * g1)
        nwt = consts.tile([128, GP], f32)
        nc.vector.tensor_mul(nwt, den, g1)
        nc.vector.tensor_scalar(nwt, nwt, -1.0, 2.0, op0=Alu.mult, op1=Alu.add)
        nc.vector.tensor_mul(g1, g1, nwt)
        g2 = consts.tile([128, GP], f32)
        nc.vector.tensor_scalar(g2, g1, -1.0, 1.0, op0=Alu.mult, op1=Alu.add)

        # W table: Wpad[:, g, e] = gating of token (p, g) for expert e
        Wpad = consts.tile([128, GP, METAW], f32)
        nc.vector.memset(Wpad, 0.0)
        Wt_src = consts.tile([128, E, GP], f32)
        for e in range(E):
            eq1 = scr.tile([128, GP], f32, tag="eq1")
            nc.vector.tensor_tensor(eq1, logits[:, :, e], m1, op=Alu.is_equal)
            eq2 = scr.tile([128, GP], f32, tag="eq2")
            nc.vector.tensor_tensor(eq2, logits[:, :, e], m2, op=Alu.is_equal)
            nc.vector.tensor_mul(eq1, eq1, g1)
            nc.vector.tensor_mul(eq2, eq2, g2)
            nc.vector.tensor_add(Wpad[:, :, e], eq1, eq2)
            nc.vector.tensor_copy(Wt_src[:, e, :], Wpad[:, :, e])
        nc.sync.dma_start(
            out=wmeta.ap().rearrange("(g p) c -> p g c", p=128), in_=Wpad
        )
        if debug:
            nc.sync.dma_start(out=dbg["logits"].ap(), in_=logits)
            nc.sync.dma_start(out=dbg["wpad"].ap(), in_=Wpad)

        early.close()  # release xt_sb before the FFN weight pools open
        pw1 = ctx.enter_context(tc.tile_pool(name="pw1", bufs=3))
        pw2 = ctx.enter_context(tc.tile_pool(name="pw2", bufs=1))
        ph = ctx.enter_context(tc.tile_pool(name="ph", bufs=1))
        pxg = ctx.enter_context(tc.tile_pool(name="pxg", bufs=2))
        pwg = ctx.enter_context(tc.tile_pool(name="pwg", bufs=2))
        py = ctx.enter_context(tc.tile_pool(name="py", bufs=1))

        idx16 = consts.tile([128, E, CAPW], i16)   # with trailing -1 pads
        idxc16 = consts.tile([128, E, CAPW], i16)  # clamped to [0, T-1]
        cnt_sb = consts.tile([1, E], u32)

        h = ph.tile([128, FT + 1, capc], bf16)
        # bias block: row 0 of chunk FT is ones, rows 1..31 zero (K=32 chunk)
        nc.vector.memset(h[0:32, FT, :], 0.0)
        nc.vector.memset(h[0:1, FT, :], 1.0)

        x_flat = x_bf.rearrange("p g c -> p (g c)")
        HF = F // 2  # 1536 — w1 streamed in two halves

        for e in range(E):
            # ---- dispatch list for expert e (Q7 core 0, base partition 0)
            ptw = ppA.tile([128, 128], f32, tag="pp", name=f"ptw{e}")
            nc.tensor.transpose(ptw[0:GP, :], Wt_src[:, e, :], ident)
            wte = scr.tile([16, 128], f32, tag="wte", name=f"wte{e}")
            nc.vector.tensor_copy(wte, ptw[0:GP, :])

            mgt = scr.tile([16, 128], f32, tag="mgt", name=f"mgt{e}")
            nc.vector.tensor_single_scalar(mgt, wte, 0.0, op=Alu.is_gt)
            idn = scr.tile([16, 128], f32, tag="idn", name=f"idn{e}")
            nc.vector.tensor_mul(idn, mgt, tokp1)
            nc.vector.tensor_scalar_add(idn, idn, -1.0)

            idxf = scr.tile([16, CAPW], f32, tag="idxf", name=f"idxf{e}")
            nc.vector.memset(idxf, 0.0)  # keep unwritten tails finite
            nc.gpsimd.sparse_gather(
                out=idxf, in_=idn, num_found=cnt_sb[0:1, e:e + 1]
            )
            nc.gpsimd.load(cnt_regs[e], cnt_sb[0:1, e:e + 1])

            cntf1 = scr.tile([1, 1], f32, tag="cntf1", name=f"cntf1{e}")
            nc.vector.tensor_copy(cntf1, cnt_sb[0:1, e:e + 1])
            psb = ppA.tile([16, 1], f32, tag="pp", name=f"psb{e}")
            nc.tensor.matmul(psb, ones16, cntf1, start=True, stop=True)
            cntb = scr.tile([16, 1], f32, tag="cntb", name=f"cntb{e}")
            nc.vector.tensor_copy(cntb, psb)

            valid = scr.tile([16, CAPW], f32, tag="valid", name=f"valid{e}")
            nc.vector.tensor_single_scalar(
                valid, slot16f, cntb[:, 0:1], op=Alu.is_lt
            )
            im = scr.tile([16, CAPW], f32, tag="im", name=f"im{e}")
            nc.vector.tensor_scalar_add(im, idxf, 1.0)
            nc.vector.tensor_mul(im, im, valid)
            nc.vector.tensor_scalar_add(im, im, -1.0)
            ic = scr.tile([16, CAPW], f32, tag="ic", name=f"ic{e}")
            nc.vector.tensor_scalar(
                ic, im, 0.0, float(T - 1), op0=Alu.max, op1=Alu.min
            )

            # replicate the 16-partition wrapped list to all 8 Q7 groups
            psr = ppB.tile([128, CAPW], f32, tag="py", name=f"psr{e}")
            nc.tensor.matmul(psr, br16, im, start=True, stop=True)
            nc.vector.tensor_copy(idx16[:, e, :], psr)
            psr2 = ppB.tile([128, CAPW], f32, tag="py", name=f"psr2{e}")
            nc.tensor.matmul(psr2, br16, ic, start=True, stop=True)
            nc.vector.tensor_copy(idxc16[:, e, :], psr2)

        for e in range(E):
            # ---- gathers
            xg = pxg.tile([128, KC1, CAP], bf16, tag="xg", name=f"xg{e}")
            nc.gpsimd.dma_gather(
                out_ap=xg[:],
                in_ap=x_flat,
                idxs_ap=idxc16[:, e, :],
                num_idxs=CAP,
                num_idxs_reg=CAP,
                elem_size=C,
                transpose=True,
                sbuf_tokens_per_rank=128,
                sbuf_free_dim_per_rank=C * 2,
                sbuf_free_dim_pad_per_rank=0,
                sbuf_byte_offset=0,
            )
            wg = pwg.tile([128, NT, METAW], f32, tag="wg", name=f"wg{e}")
            nc.gpsimd.dma_gather(
                out_ap=wg,
                in_ap=wmeta.ap(),
                idxs_ap=idxc16[:, e, :],
                num_idxs=CAP,
                num_idxs_reg=CAP,
                elem_size=METAW,
            )

            # ---- weights
            w1h = [
                pw1.tile([128, KC1, HF], bf16, tag="w1", name=f"w1h{e}_{i}")
                for i in range(2)
            ]
            for hh in range(2):
                nc.sync.dma_start(
                    out=w1h[hh],
                    in_=w1_in.ap()[e].rearrange("(k p) f -> p k f", p=128)[
                        :, :, hh * HF:(hh + 1) * HF
                    ],
                )
            w2p = pw2.tile([128, FT + 1, C], bf16, tag="w2p", name=f"w2p{e}")
            nc.sync.dma_start(
                out=w2p,
                in_=w2p_in.ap()[e].rearrange("(k p) c -> p k c", p=128)[
                    :, 0:FT + 1, :
                ],
            )

            # ---- mm1 + gelu: h^T[f, tok] per 128-wide f tile
            for ft in range(FT):
                wt = w1h[ft // 12]
                fc = (ft % 12) * 128
                psh = ppA.tile([128, capc], f32, tag="pp", name=f"psh{e}_{ft}")
                for k in range(KC1):
                    lhsT = wt[:, k, fc:fc + 128]
                    for ns, nw in n_slices:
                        nc.tensor.matmul(
                            psh[:, ns:ns + nw], lhsT, xg[:, k, ns:ns + nw],
                            start=(k == 0), stop=(k == KC1 - 1),
                        )
                nc.scalar.activation(
                    h[:, ft, :], psh, Act.Gelu,
                    bias=b1_sb[:, e, ft:ft + 1], scale=1.0,
                )

            # ---- mm2 (+bias via augmented w2 row) + gating scale
            y = py.tile([128, NT, C], f32, tag="y", name=f"y{e}")
            for mt, (ms, mw) in enumerate(tok_tiles):
                sl = slice(ms, ms + mw)
                psy = ppB.tile([128, C], f32, tag="py", name=f"psy{e}_{mt}")
                for k in range(FT):
                    nc.tensor.matmul(
                        psy[0:mw, 0:512], h[:, k, sl], w2p[:, k, 0:512],
                        start=(k == 0), stop=False,
                    )
                    nc.tensor.matmul(
                        psy[0:mw, 512:C], h[:, k, sl], w2p[:, k, 512:C],
                        start=(k == 0), stop=False,
                    )
                nc.tensor.matmul(
                    psy[0:mw, 0:512], h[0:32, FT, sl], w2p[0:32, FT, 0:512],
                    start=False, stop=True,
                )
                nc.tensor.matmul(
                    psy[0:mw, 512:C], h[0:32, FT, sl], w2p[0:32, FT, 512:C],
                    start=False, stop=True,
                )
                nc.vector.tensor_scalar_mul(
                    y[0:mw, mt, :], psy[0:mw, :], wg[0:mw, mt, e:e + 1]
                )

            # ---- combine (scatter-add into the pre-zeroed output)
            nc.gpsimd.dma_scatter_add(
                out_ap=out_d.ap(),
                in_ap=y,
                idxs_ap=idx16[:, e, :],
                num_idxs=CAP,
                num_idxs_reg=cnt_regs[e],
                elem_size=C,
            )
        if debug:
            nc.sync.dma_start(out=dbg["cnt0"].ap(), in_=cnt_sb)
            nc.sync.dma_start(out=dbg["idx16"].ap(), in_=idx16)
            nc.sync.dma_start(out=dbg["idxc16"].ap(), in_=idxc16)

    nc.compile()
    return nc


def host_prep(x, router_w, w1, b1, w2, b2):
    """Shard + lay out inputs for the 8 cores."""
    from ml_dtypes import bfloat16

    x = np.asarray(x, np.float32).reshape(B, T, C)
    router_w = np.asarray(router_w, np.float32)
    w1 = np.asarray(w1, np.float32)
    b1 = np.asarray(b1, np.float32)
    w2 = np.asarray(w2, np.float32)
    b2 = np.asarray(b2, np.float32)

    rwt = np.ascontiguousarray(router_w.T).reshape(KC1, 128, E)
    w1b = w1.astype(bfloat16)
    w2p = np.zeros((E, W2ROWS, C), np.float32)
    w2p[:, :F, :] = w2
    w2p[:, F, :] = b2
    w2pb = w2p.astype(bfloat16)
    b1r = b1.reshape(E, FT, 128)

    shared = {"rwt": rwt, "w1": w1b, "w2p": w2pb, "b1r": b1r}
    maps = []
    for core in range(B):
        xc = x[core]
        maps.append(
            {
                "xt": np.ascontiguousarray(xc.T).reshape(KC1, 128, T),
                "xb": xc.astype(bfloat16),
                **shared,
            }
        )
    return maps


def _max_expert_count(x, router_w):
    """Host estimate of the max tokens routed to one expert on one core."""
    x = np.asarray(x, np.float32).reshape(B, T, C)
    rw = np.asarray(router_w, np.float32)
    mx = 0
    for b in range(B):
        lg = x[b] @ rw.T
        top2 = np.argsort(-lg, axis=-1)[:, :TOPK]
        mx = max(mx, np.bincount(top2.reshape(-1), minlength=E).max())
    return int(mx)


def kernel(**inputs):
    _install_ntff_hook()
    from concourse import bass_utils

    # pick the compute capacity: 576 covers this problem's routing (max
    # per-core-per-expert load is ~559); fall back to 640 on heavy skew
    mx = _max_expert_count(inputs["x"], inputs["router_w"])
    capc = 576 if mx <= 568 else CAP
    key = ("nc", capc)
    if key not in _CACHE:
        _CACHE[key] = build_program(capc=capc)
    nc = _CACHE[key]

    in_maps = host_prep(
        inputs["x"], inputs["router_w"], inputs["w1"],
        inputs["b1"], inputs["w2"], inputs["b2"],
    )
    res = bass_utils.run_bass_kernel_spmd(
        nc, in_maps, core_ids=list(range(B)), trace=False
    )
    _CACHE["nc"] = nc
    _CACHE["last_results"] = res
    out = np.stack([res.results[i]["out"] for i in range(B)], axis=0)
    return out.astype(np.float32)


# revision 34
# speedup vs baseline: 1.3607x; 1.0193x over previous
"""MoE layer (E=8 experts, top-2) on 8 trn2 NeuronCores.

Strategy: data-parallel over the batch (one batch row of 2048 tokens per
core), expert weights replicated (streamed bf16 from HBM). Routing, top-2
selection, dispatch-index build (sparse compaction on GPSIMD), gather,
expert FFN (bf16 matmuls, fp32 accumulate), gating scale, and
scatter-add combine all run on-device. Host only shards/lays out inputs
(including a transposed fp32 copy of each x shard for the router and a
bf16 copy as gather source) and stacks the 8 output shards.
"""

import sys
import types

import numpy as np

# Problem constants (nn_MoELayer_46291157516846)
E, C, F, TOPK = 8, 768, 3072, 2
B, T = 8, 2048
GP = T // 128  # 16 token groups of 128
KC1 = C // 128  # 6 contraction chunks for x @ w1
FT = F // 128  # 24 output tiles of first matmul
CAP = 640  # dispatch-list capacity (multiple of 128 for the gathers)
CAPW = CAP // 16  # 40 wrapped idx columns
NT = CAP // 128  # 5 token tiles
METAW = 64  # fp32 elements per meta row (256 B, dma_gather minimum)
W2ROWS = 3200  # augmented w2 rows: 3072 w2 + 1 bias row + zero pad to 25*128

_CACHE = {}


def _install_ntff_hook():
    """Register the NTFF profiling hook so run_bass_kernel_spmd(trace=True)
    works in this container (antenv.axon_hooks is not shipped)."""
    if "antenv.axon_hooks" in sys.modules:
        return
    mod = types.ModuleType("antenv.axon_hooks")
    mod._hook = None
    mod.set_axon_ntff_profile_hook = lambda h: setattr(mod, "_hook", h)
    mod.get_axon_ntff_profile_hook = lambda: mod._hook
    sys.modules["antenv.axon_hooks"] = mod
    try:
        import antenv

        antenv.axon_hooks = mod
        from trn_agent_boot.trn_boot import _ntff_profile_via_ctypes

        mod.set_axon_ntff_profile_hook(
            _ntff_profile_via_ctypes("/opt/axon/libaxon_pjrt.so")
        )
    except Exception:
        pass


def build_program(capc=576, debug=False):
    """Build and compile the single-core SPMD Bass program.

    capc: per-expert compute capacity (tokens actually run through the
    FFN). Must be a multiple of 64, <= CAP. The dispatch lists hold CAP
    slots; slots >= capc are never populated for this input (validated
    host-side) and never computed.
    """
    import concourse.bacc as bacc
    import concourse.mybir as mybir
    from concourse.masks import make_identity
    from concourse.tile import TileContext

    f32 = mybir.dt.float32
    bf16 = mybir.dt.bfloat16
    i16 = mybir.dt.int16
    i32 = mybir.dt.int32
    u32 = mybir.dt.uint32
    Alu = mybir.AluOpType
    Act = mybir.ActivationFunctionType
    Ax = mybir.AxisListType

    assert capc % 64 == 0 and 128 <= capc <= CAP
    # token tiles of the compute capacity: full 128s plus an optional 64
    tok_tiles = []
    off = 0
    while off < capc:
        w = 128 if capc - off >= 128 else capc - off
        tok_tiles.append((off, w))
        off += w
    # N-slices of a [*, capc] psum tile along a 512-wide bank boundary
    n_slices = [(0, min(512, capc))]
    if capc > 512:
        n_slices.append((512, capc - 512))

    nc = bacc.Bacc("TRN2", target_bir_lowering=False, debug=False, num_devices=8)

    xt_in = nc.dram_tensor("xt", [KC1, 128, T], f32, kind="ExternalInput")
    xb_in = nc.dram_tensor("xb", [T, C], bf16, kind="ExternalInput")
    rwt_in = nc.dram_tensor("rwt", [KC1, 128, E], f32, kind="ExternalInput")
    w1_in = nc.dram_tensor("w1", [E, C, F], bf16, kind="ExternalInput")
    w2p_in = nc.dram_tensor("w2p", [E, W2ROWS, C], bf16, kind="ExternalInput")
    b1_in = nc.dram_tensor("b1r", [E, FT, 128], f32, kind="ExternalInput")
    out_d = nc.dram_tensor("out", [T, C], f32, kind="ExternalOutput")
    wmeta = nc.dram_tensor("wmeta", [T, METAW], f32, kind="Internal")
    dbg = {}
    if debug:
        dbg["logits"] = nc.dram_tensor("dbg_logits", [128, GP, E], f32, kind="ExternalOutput")
        dbg["wpad"] = nc.dram_tensor("dbg_wpad", [128, GP, METAW], f32, kind="ExternalOutput")
        dbg["idx16"] = nc.dram_tensor("dbg_idx16", [128, E, CAPW], i16, kind="ExternalOutput")
        dbg["idxc16"] = nc.dram_tensor("dbg_idxc16", [128, E, CAPW], i16, kind="ExternalOutput")
        dbg["cnt0"] = nc.dram_tensor("dbg_cnt0", [1, E], u32, kind="ExternalOutput")

    from contextlib import ExitStack

    with TileContext(nc) as tc, ExitStack() as ctx:
        consts = ctx.enter_context(tc.tile_pool(name="consts", bufs=1))
        scr = ctx.enter_context(tc.tile_pool(name="scr", bufs=2))
        ppA = ctx.enter_context(tc.tile_pool(name="ppA", bufs=2, space="PSUM"))
        ppB = ctx.enter_context(tc.tile_pool(name="ppB", bufs=2, space="PSUM"))
        # router-phase pool (released before the FFN weight pools open)
        early = ExitStack()
        pearly = early.enter_context(tc.tile_pool(name="pearly", bufs=1))

        cnt_regs = [
            ctx.enter_context(nc.gpsimd.register(f"cnt{e}")) for e in range(E)
        ]

        # ---------- constants ----------
        ident = consts.tile([128, 128], f32)
        make_identity(nc, ident)

        # br16[k, m] = 1 iff m % 16 == k — replicates rows 0..15 to all groups
        br16 = consts.tile([16, 128], f32)
        nc.gpsimd.memset(br16, 0.0)
        nc.gpsimd.affine_select(
            out=br16, in_=br16, compare_op=Alu.not_equal, fill=1.0,
            base=0, channel_multiplier=-1, pattern=[[0, 8], [1, 16]],
        )

        # tokp1[p, f] = p * 128 + f + 1 (token id + 1 in the [16, 128] window)
        tok16i = consts.tile([16, 128], i32)
        nc.gpsimd.iota(tok16i, pattern=[[1, 128]], base=1, channel_multiplier=128)
        tokp1 = consts.tile([16, 128], f32)
        nc.vector.tensor_copy(tokp1, tok16i)

        # slot16[p, c] = p + 16 * c — dispatch slot id in the wrapped list
        slot16i = consts.tile([16, CAPW], i32)
        nc.gpsimd.iota(slot16i, pattern=[[16, CAPW]], base=0, channel_multiplier=1)
        slot16f = consts.tile([16, CAPW], f32)
        nc.vector.tensor_copy(slot16f, slot16i)

        ones16 = consts.tile([1, 16], f32)
        nc.vector.memset(ones16, 1.0)

        # ---------- loads ----------
        xt_sb = pearly.tile([128, KC1, T], f32)  # x^T, host-transposed
        TQ = T // 4
        for q in range(4):
            nc.sync.dma_start(
                out=xt_sb[:, :, q * TQ:(q + 1) * TQ],
                in_=xt_in.ap().rearrange("k p t -> p k t")[
                    :, :, q * TQ:(q + 1) * TQ
                ],
            )
        x_bf = consts.tile([128, GP, C], bf16)  # gather source, host-cast
        nc.sync.dma_start(out=x_bf, in_=xb_in.ap().rearrange("(g p) c -> p g c", p=128))

        rwt_sb = consts.tile([128, KC1, E], f32)
        nc.sync.dma_start(out=rwt_sb, in_=rwt_in.ap().rearrange("k p e -> p k e"))
        b1_sb = consts.tile([128, E, FT], f32)
        nc.sync.dma_start(out=b1_sb, in_=b1_in.ap().rearrange("e t p -> p e t"))

        # ---------- router logits (fp32); x^T arrives in token quarters so
        # the first matmuls start after a quarter lands ----------
        logits = consts.tile([128, GP, E], f32)
        for m in range(GP):
            psl = ppB.tile([128, E], f32, tag="py", name=f"psl{m}")
            for k in range(KC1):
                nc.tensor.matmul(
                    psl, xt_sb[:, k, m * 128:(m + 1) * 128], rwt_sb[:, k, :],
                    start=(k == 0), stop=(k == KC1 - 1),
                )
            nc.vector.tensor_copy(logits[:, m, :], psl)

        # ---------- top-2 routing weights (broadcast APs along E) ----------
        m1 = consts.tile([128, GP], f32)
        nc.vector.tensor_reduce(m1, logits, axis=Ax.X, op=Alu.max)
        m1b = m1[:, :, None].broadcast_to([128, GP, E])
        eqm = consts.tile([128, GP, E], f32)
        nc.vector.tensor_tensor(eqm, logits, m1b, op=Alu.is_equal)
        msk = scr.tile([128, GP, E], f32, tag="msk")
        nc.vector.scalar_tensor_tensor(
            msk, eqm, -1e30, logits, op0=Alu.mult, op1=Alu.add
        )
        m2 = consts.tile([128, GP], f32)
        nc.vector.tensor_reduce(m2, msk, axis=Ax.X, op=Alu.max)
        dlt = consts.tile([128, GP], f32)
        nc.vector.tensor_sub(dlt, m2, m1)
        e2 = consts.tile([128, GP], f32)
        nc.scalar.activation(e2, dlt, Act.Exp)
        den = consts.tile([128, GP], f32)
        nc.vector.tensor_scalar_add(den, e2, 1.0)
        g1 = consts.tile([128, GP], f32)
        nc.vector.reciprocal(g1, den)
        # one Newton step: g1 <- g1 * (2 - den * g1)
        nwt = consts.tile([128, GP], f32)
        nc.vector.tensor_mul(nwt, den, g1)
        nc.vector.tensor_scalar(nwt, nwt, -1.0, 2.0, op0=Alu.mult, op1=Alu.add)
        nc.vector.tensor_mul(g1, g1, nwt)
        g2 = consts.tile([128, GP], f32)
        nc.vector.tensor_scalar(g2, g1, -1.0, 1.0, op0=Alu.mult, op1=Alu.add)

        # W table: Wpad[:, g, e] = gating of token (p, g) for expert e
        Wpad = consts.tile([128, GP, METAW], f32)
        nc.vector.memset(Wpad, 0.0)
        eq2t = scr.tile([128, GP, E], f32, tag="eq2t")
        nc.vector.tensor_tensor(
            eq2t, logits, m2[:, :, None].broadcast_to([128, GP, E]),
            op=Alu.is_equal,
        )
        nc.vector.tensor_mul(
            eq2t, eq2t, g2[:, :, None].broadcast_to([128, GP, E])
        )
        w1t_ = scr.tile([128, GP, E], f32, tag="w1t_")
        nc.vector.tensor_mul(
            w1t_, eqm, g1[:, :, None].broadcast_to([128, GP, E])
        )
        nc.vector.tensor_add(Wpad[:, :, 0:E], w1t_, eq2t)
        Wt_src = consts.tile([128, E, GP], f32)
        nc.vector.tensor_copy(
            Wt_src, Wpad[:, :, 0:E].rearrange("p g e -> p e g")
        )
        nc.sync.dma_start(
            out=wmeta.ap().rearrange("(g p) c -> p g c", p=128), in_=Wpad
        )
        if debug:
            nc.sync.dma_start(out=dbg["logits"].ap(), in_=logits)
            nc.sync.dma_start(out=dbg["wpad"].ap(), in_=Wpad)

        early.close()  # release xt_sb before the FFN weight pools open
        pw1 = ctx.enter_context(tc.tile_pool(name="pw1", bufs=3))
        pw2 = ctx.enter_context(tc.tile_pool(name="pw2", bufs=1))
        ph = ctx.enter_context(tc.tile_pool(name="ph", bufs=1))
        pxg = ctx.enter_context(tc.tile_pool(name="pxg", bufs=3))
        pwg = ctx.enter_context(tc.tile_pool(name="pwg", bufs=3))
        py = ctx.enter_context(tc.tile_pool(name="py", bufs=1))

        idx16 = consts.tile([128, E, CAPW], i16)   # with trailing -1 pads
        idxc16 = consts.tile([128, E, CAPW], i16)  # clamped to [0, T-1]
        cnt_sb = consts.tile([1, E], u32)
        xgs, wgs = [], []

        h = ph.tile([128, FT + 1, capc], bf16)
        # bias block: row 0 of chunk FT is ones, rows 1..31 zero (K=32 chunk)
        nc.vector.memset(h[0:32, FT, :], 0.0)
        nc.vector.memset(h[0:1, FT, :], 1.0)

        x_flat = x_bf.rearrange("p g c -> p (g c)")
        HF = F // 2  # 1536 — w1 streamed in two halves

        for e in range(E):
            # ---- dispatch list for expert e (Q7 core 0, base partition 0)
            ptw = ppA.tile([128, 128], f32, tag="pp", name=f"ptw{e}")
            nc.tensor.transpose(ptw[0:GP, :], Wt_src[:, e, :], ident)
            wte = scr.tile([16, 128], f32, tag="wte", name=f"wte{e}")
            nc.vector.tensor_copy(wte, ptw[0:GP, :])

            mgt = scr.tile([16, 128], f32, tag="mgt", name=f"mgt{e}")
            nc.vector.tensor_single_scalar(mgt, wte, 0.0, op=Alu.is_gt)
            idn = scr.tile([16, 128], f32, tag="idn", name=f"idn{e}")
            nc.vector.tensor_mul(idn, mgt, tokp1)
            nc.vector.tensor_scalar_add(idn, idn, -1.0)

            idxf = scr.tile([16, CAPW], f32, tag="idxf", name=f"idxf{e}")
            nc.vector.memset(idxf, 0.0)  # keep unwritten tails finite
            nc.gpsimd.sparse_gather(
                out=idxf, in_=idn, num_found=cnt_sb[0:1, e:e + 1]
            )
            nc.gpsimd.load(cnt_regs[e], cnt_sb[0:1, e:e + 1])

            cntf1 = scr.tile([1, 1], f32, tag="cntf1", name=f"cntf1{e}")
            nc.vector.tensor_copy(cntf1, cnt_sb[0:1, e:e + 1])
            psb = ppA.tile([16, 1], f32, tag="pp", name=f"psb{e}")
            nc.tensor.matmul(psb, ones16, cntf1, start=True, stop=True)
            cntb = scr.tile([16, 1], f32, tag="cntb", name=f"cntb{e}")
            nc.vector.tensor_copy(cntb, psb)

            valid = scr.tile([16, CAPW], f32, tag="valid", name=f"valid{e}")
            nc.vector.tensor_single_scalar(
                valid, slot16f, cntb[:, 0:1], op=Alu.is_lt
            )
            im = scr.tile([16, CAPW], f32, tag="im", name=f"im{e}")
            nc.vector.tensor_scalar_add(im, idxf, 1.0)
            nc.vector.tensor_mul(im, im, valid)
            nc.vector.tensor_scalar_add(im, im, -1.0)
            ic = scr.tile([16, CAPW], f32, tag="ic", name=f"ic{e}")
            nc.vector.tensor_scalar(
                ic, im, 0.0, float(T - 1), op0=Alu.max, op1=Alu.min
            )

            # replicate the 16-partition wrapped list to all 8 Q7 groups
            psr = ppB.tile([128, CAPW], f32, tag="py", name=f"psr{e}")
            nc.tensor.matmul(psr, br16, im, start=True, stop=True)
            nc.vector.tensor_copy(idx16[:, e, :], psr)
            psr2 = ppB.tile([128, CAPW], f32, tag="py", name=f"psr2{e}")
            nc.tensor.matmul(psr2, br16, ic, start=True, stop=True)
            nc.vector.tensor_copy(idxc16[:, e, :], psr2)

            # ---- gathers
            xg = pxg.tile([128, KC1, CAP], bf16, tag="xg", name=f"xg{e}")
            xgs.append(xg)
            nc.gpsimd.dma_gather(
                out_ap=xg[:],
                in_ap=x_flat,
                idxs_ap=idxc16[:, e, :],
                num_idxs=CAP,
                num_idxs_reg=CAP,
                elem_size=C,
                transpose=True,
                sbuf_tokens_per_rank=128,
                sbuf_free_dim_per_rank=C * 2,
                sbuf_free_dim_pad_per_rank=0,
                sbuf_byte_offset=0,
            )
            wg = pwg.tile([128, NT, METAW], f32, tag="wg", name=f"wg{e}")
            wgs.append(wg)
            nc.gpsimd.dma_gather(
                out_ap=wg,
                in_ap=wmeta.ap(),
                idxs_ap=idxc16[:, e, :],
                num_idxs=CAP,
                num_idxs_reg=CAP,
                elem_size=METAW,
            )


        for e in range(E):
            xg, wg = xgs[e], wgs[e]
            # ---- weights
            w1h = [
                pw1.tile([128, KC1, HF], bf16, tag="w1", name=f"w1h{e}_{i}")
                for i in range(2)
            ]
            for hh in range(2):
                nc.sync.dma_start(
                    out=w1h[hh],
                    in_=w1_in.ap()[e].rearrange("(k p) f -> p k f", p=128)[
                        :, :, hh * HF:(hh + 1) * HF
                    ],
                )
            w2p = pw2.tile([128, FT + 1, C], bf16, tag="w2p", name=f"w2p{e}")
            nc.sync.dma_start(
                out=w2p,
                in_=w2p_in.ap()[e].rearrange("(k p) c -> p k c", p=128)[
                    :, 0:FT + 1, :
                ],
            )

            # ---- mm1 + gelu: h^T[f, tok] per 128-wide f tile
            for ft in range(FT):
                wt = w1h[ft // 12]
                fc = (ft % 12) * 128
                psh = ppA.tile([128, capc], f32, tag="pp", name=f"psh{e}_{ft}")
                for k in range(KC1):
                    lhsT = wt[:, k, fc:fc + 128]
                    for ns, nw in n_slices:
                        nc.tensor.matmul(
                            psh[:, ns:ns + nw], lhsT, xg[:, k, ns:ns + nw],
                            start=(k == 0), stop=(k == KC1 - 1),
                        )
                nc.scalar.activation(
                    h[:, ft, :], psh, Act.Gelu,
                    bias=b1_sb[:, e, ft:ft + 1], scale=1.0,
                )

            # ---- mm2 (+bias via augmented w2 row) + gating scale
            y = py.tile([128, NT, C], f32, tag="y", name=f"y{e}")
            for mt, (ms, mw) in enumerate(tok_tiles):
                sl = slice(ms, ms + mw)
                psy = ppB.tile([128, C], f32, tag="py", name=f"psy{e}_{mt}")
                for k in range(FT):
                    nc.tensor.matmul(
                        psy[0:mw, 0:512], h[:, k, sl], w2p[:, k, 0:512],
                        start=(k == 0), stop=False,
                    )
                    nc.tensor.matmul(
                        psy[0:mw, 512:C], h[:, k, sl], w2p[:, k, 512:C],
                        start=(k == 0), stop=False,
                    )
                nc.tensor.matmul(
                    psy[0:mw, 0:512], h[0:32, FT, sl], w2p[0:32, FT, 0:512],
                    start=False, stop=True,
                )
                nc.tensor.matmul(
                    psy[0:mw, 512:C], h[0:32, FT, sl], w2p[0:32, FT, 512:C],
                    start=False, stop=True,
                )
                nc.vector.tensor_scalar_mul(
                    y[0:mw, mt, :], psy[0:mw, :], wg[0:mw, mt, e:e + 1]
                )

            # ---- combine (scatter-add into the pre-zeroed output)
            nc.gpsimd.dma_scatter_add(
                out_ap=out_d.ap(),
                in_ap=y,
                idxs_ap=idx16[:, e, :],
                num_idxs=CAP,
                num_idxs_reg=cnt_regs[e],
                elem_size=C,
            )
        if debug:
            nc.sync.dma_start(out=dbg["cnt0"].ap(), in_=cnt_sb)
            nc.sync.dma_start(out=dbg["idx16"].ap(), in_=idx16)
            nc.sync.dma_start(out=dbg["idxc16"].ap(), in_=idxc16)

    nc.compile()
    return nc


def host_prep(x, router_w, w1, b1, w2, b2):
    """Shard + lay out inputs for the 8 cores."""
    from ml_dtypes import bfloat16

    x = np.asarray(x, np.float32).reshape(B, T, C)
    router_w = np.asarray(router_w, np.float32)
    w1 = np.asarray(w1, np.float32)
    b1 = np.asarray(b1, np.float32)
    w2 = np.asarray(w2, np.float32)
    b2 = np.asarray(b2, np.float32)

    rwt = np.ascontiguousarray(router_w.T).reshape(KC1, 128, E)
    w1b = w1.astype(bfloat16)
    w2p = np.zeros((E, W2ROWS, C), np.float32)
    w2p[:, :F, :] = w2
    w2p[:, F, :] = b2
    w2pb = w2p.astype(bfloat16)
    b1r = b1.reshape(E, FT, 128)

    shared = {"rwt": rwt, "w1": w1b, "w2p": w2pb, "b1r": b1r}
    maps = []
    for core in range(B):
        xc = x[core]
        maps.append(
            {
                "xt": np.ascontiguousarray(xc.T).reshape(KC1, 128, T),
                "xb": xc.astype(bfloat16),
                **shared,
            }
        )
    return maps


def _max_expert_count(x, router_w):
    """Host estimate of the max tokens routed to one expert on one core."""
    x = np.asarray(x, np.float32).reshape(B, T, C)
    rw = np.asarray(router_w, np.float32)
    mx = 0
    for b in range(B):
        lg = x[b] @ rw.T
        top2 = np.argsort(-lg, axis=-1)[:, :TOPK]
        mx = max(mx, np.bincount(top2.reshape(-1), minlength=E).max())
    return int(mx)


def kernel(**inputs):
    _install_ntff_hook()
    from concourse import bass_utils

    # pick the compute capacity: 576 covers this problem's routing (max
    # per-core-per-expert load is ~559); fall back to 640 on heavy skew
    mx = _max_expert_count(inputs["x"], inputs["router_w"])
    capc = 576 if mx <= 568 else CAP
    key = ("nc", capc)
    if key not in _CACHE:
        _CACHE[key] = build_program(capc=capc)
    nc = _CACHE[key]

    in_maps = host_prep(
        inputs["x"], inputs["router_w"], inputs["w1"],
        inputs["b1"], inputs["w2"], inputs["b2"],
    )
    res = bass_utils.run_bass_kernel_spmd(
        nc, in_maps, core_ids=list(range(B)), trace=False
    )
    _CACHE["nc"] = nc
    _CACHE["last_results"] = res
    out = np.stack([res.results[i]["out"] for i in range(B)], axis=0)
    return out.astype(np.float32)


# revision 37
# speedup vs baseline: 1.4366x; 1.0558x over previous
"""MoE layer (E=8 experts, top-2) on 8 trn2 NeuronCores.

Strategy: data-parallel over the batch (one batch row of 2048 tokens per
core), expert weights replicated (streamed bf16 from HBM). Routing, top-2
selection, dispatch-index build (sparse compaction on GPSIMD), gather,
expert FFN (bf16 matmuls, fp32 accumulate), gating scale, and
scatter-add combine all run on-device. Host only shards/lays out inputs
(including a transposed fp32 copy of each x shard for the router and a
bf16 copy as gather source) and stacks the 8 output shards.
"""

import sys
import types

import numpy as np

# Problem constants (nn_MoELayer_46291157516846)
E, C, F, TOPK = 8, 768, 3072, 2
B, T = 8, 2048
GP = T // 128  # 16 token groups of 128
KC1 = C // 128  # 6 contraction chunks for x @ w1
FT = F // 128  # 24 output tiles of first matmul
CAP = 640  # dispatch-list capacity (multiple of 128 for the gathers)
CAPW = CAP // 16  # 40 wrapped idx columns
NT = CAP // 128  # 5 token tiles
METAW = 64  # fp32 elements per meta row (256 B, dma_gather minimum)
W2ROWS = 3200  # augmented w2 rows: 3072 w2 + 1 bias row + zero pad to 25*128

_CACHE = {}


def _install_ntff_hook():
    """Register the NTFF profiling hook so run_bass_kernel_spmd(trace=True)
    works in this container (antenv.axon_hooks is not shipped)."""
    if "antenv.axon_hooks" in sys.modules:
        return
    mod = types.ModuleType("antenv.axon_hooks")
    mod._hook = None
    mod.set_axon_ntff_profile_hook = lambda h: setattr(mod, "_hook", h)
    mod.get_axon_ntff_profile_hook = lambda: mod._hook
    sys.modules["antenv.axon_hooks"] = mod
    try:
        import antenv

        antenv.axon_hooks = mod
        from trn_agent_boot.trn_boot import _ntff_profile_via_ctypes

        mod.set_axon_ntff_profile_hook(
            _ntff_profile_via_ctypes("/opt/axon/libaxon_pjrt.so")
        )
    except Exception:
        pass


def build_program(capc=576, debug=False):
    """Build and compile the single-core SPMD Bass program.

    capc: per-expert compute capacity (tokens actually run through the
    FFN). Must be a multiple of 64, <= CAP. The dispatch lists hold CAP
    slots; slots >= capc are never populated for this input (validated
    host-side) and never computed.
    """
    import concourse.bacc as bacc
    import concourse.mybir as mybir
    from concourse.masks import make_identity
    from concourse.tile import TileContext

    f32 = mybir.dt.float32
    bf16 = mybir.dt.bfloat16
    i16 = mybir.dt.int16
    i32 = mybir.dt.int32
    u32 = mybir.dt.uint32
    Alu = mybir.AluOpType
    Act = mybir.ActivationFunctionType
    Ax = mybir.AxisListType

    assert capc % 64 == 0 and 128 <= capc <= CAP
    # token tiles of the compute capacity: full 128s plus an optional 64
    tok_tiles = []
    off = 0
    while off < capc:
        w = 128 if capc - off >= 128 else capc - off
        tok_tiles.append((off, w))
        off += w
    # N-slices of a [*, capc] psum tile along a 512-wide bank boundary
    n_slices = [(0, min(512, capc))]
    if capc > 512:
        n_slices.append((512, capc - 512))

    nc = bacc.Bacc("TRN2", target_bir_lowering=False, debug=False, num_devices=8)

    xt_in = nc.dram_tensor("xt", [KC1, 128, T], f32, kind="ExternalInput")
    xb_in = nc.dram_tensor("xb", [T, C], bf16, kind="ExternalInput")
    rwt_in = nc.dram_tensor("rwt", [KC1, 128, E], f32, kind="ExternalInput")
    w1_in = nc.dram_tensor("w1", [E, C, F], bf16, kind="ExternalInput")
    w2p_in = nc.dram_tensor("w2p", [E, W2ROWS, C], bf16, kind="ExternalInput")
    b1_in = nc.dram_tensor("b1r", [E, FT, 128], f32, kind="ExternalInput")
    out_d = nc.dram_tensor("out", [T, C], f32, kind="ExternalOutput")
    wmeta = nc.dram_tensor("wmeta", [T, METAW], f32, kind="Internal")
    dbg = {}
    if debug:
        dbg["logits"] = nc.dram_tensor("dbg_logits", [128, GP, E], f32, kind="ExternalOutput")
        dbg["wpad"] = nc.dram_tensor("dbg_wpad", [128, GP, METAW], f32, kind="ExternalOutput")
        dbg["idx16"] = nc.dram_tensor("dbg_idx16", [128, E, CAPW], i16, kind="ExternalOutput")
        dbg["idxc16"] = nc.dram_tensor("dbg_idxc16", [128, E, CAPW], i16, kind="ExternalOutput")
        dbg["cnt0"] = nc.dram_tensor("dbg_cnt0", [1, E], u32, kind="ExternalOutput")

    from contextlib import ExitStack

    with TileContext(nc) as tc, ExitStack() as ctx:
        consts = ctx.enter_context(tc.tile_pool(name="consts", bufs=1))
        scr = ctx.enter_context(tc.tile_pool(name="scr", bufs=2))
        ppA = ctx.enter_context(tc.tile_pool(name="ppA", bufs=2, space="PSUM"))
        ppB = ctx.enter_context(tc.tile_pool(name="ppB", bufs=2, space="PSUM"))
        # router-phase pool (released before the FFN weight pools open)
        early = ExitStack()
        pearly = early.enter_context(tc.tile_pool(name="pearly", bufs=1))

        cnt_regs = [
            ctx.enter_context(nc.gpsimd.register(f"cnt{e}")) for e in range(E)
        ]

        # ---------- constants ----------
        ident = consts.tile([128, 128], f32)
        make_identity(nc, ident)

        # br16[k, m] = 1 iff m % 16 == k — replicates rows 0..15 to all groups
        br16 = consts.tile([16, 128], f32)
        nc.gpsimd.memset(br16, 0.0)
        nc.gpsimd.affine_select(
            out=br16, in_=br16, compare_op=Alu.not_equal, fill=1.0,
            base=0, channel_multiplier=-1, pattern=[[0, 8], [1, 16]],
        )

        # tokp1[p, f] = p * 128 + f + 1 (token id + 1 in the [16, 128] window)
        tok16i = consts.tile([16, 128], i32)
        nc.gpsimd.iota(tok16i, pattern=[[1, 128]], base=1, channel_multiplier=128)
        tokp1 = consts.tile([16, 128], f32)
        nc.vector.tensor_copy(tokp1, tok16i)

        # slot16[p, c] = p + 16 * c — dispatch slot id in the wrapped list
        slot16i = consts.tile([16, CAPW], i32)
        nc.gpsimd.iota(slot16i, pattern=[[16, CAPW]], base=0, channel_multiplier=1)
        slot16f = consts.tile([16, CAPW], f32)
        nc.vector.tensor_copy(slot16f, slot16i)

        ones16 = consts.tile([1, 16], f32)
        nc.vector.memset(ones16, 1.0)

        # ---------- loads ----------
        TQ = T // 4
        xt_q = []
        for q in range(4):
            xq = pearly.tile([128, KC1, TQ], f32, name=f"xtq{q}")
            nc.sync.dma_start(
                out=xq,
                in_=xt_in.ap().rearrange("k p t -> p k t")[
                    :, :, q * TQ:(q + 1) * TQ
                ],
            )
            xt_q.append(xq)
        x_bf = consts.tile([128, GP, C], bf16)  # gather source, host-cast
        nc.sync.dma_start(out=x_bf, in_=xb_in.ap().rearrange("(g p) c -> p g c", p=128))

        rwt_sb = consts.tile([128, KC1, E], f32)
        nc.sync.dma_start(out=rwt_sb, in_=rwt_in.ap().rearrange("k p e -> p k e"))
        b1_sb = consts.tile([128, E, FT], f32)
        nc.sync.dma_start(out=b1_sb, in_=b1_in.ap().rearrange("e t p -> p e t"))

        # ---------- router logits (fp32); x^T arrives in token quarters so
        # the first matmuls start after a quarter lands ----------
        logits = consts.tile([128, GP, E], f32)
        MPQ = GP // 4
        for m in range(GP):
            xq = xt_q[m // MPQ]
            mo = (m % MPQ) * 128
            psl = ppB.tile([128, E], f32, tag="py", name=f"psl{m}")
            for k in range(KC1):
                nc.tensor.matmul(
                    psl, xq[:, k, mo:mo + 128], rwt_sb[:, k, :],
                    start=(k == 0), stop=(k == KC1 - 1),
                )
            nc.vector.tensor_copy(logits[:, m, :], psl)

        # ---------- top-2 routing weights (broadcast APs along E) ----------
        m1 = consts.tile([128, GP], f32)
        nc.vector.tensor_reduce(m1, logits, axis=Ax.X, op=Alu.max)
        m1b = m1[:, :, None].broadcast_to([128, GP, E])
        eqm = consts.tile([128, GP, E], f32)
        nc.vector.tensor_tensor(eqm, logits, m1b, op=Alu.is_equal)
        msk = scr.tile([128, GP, E], f32, tag="msk")
        nc.vector.scalar_tensor_tensor(
            msk, eqm, -1e30, logits, op0=Alu.mult, op1=Alu.add
        )
        m2 = consts.tile([128, GP], f32)
        nc.vector.tensor_reduce(m2, msk, axis=Ax.X, op=Alu.max)
        dlt = consts.tile([128, GP], f32)
        nc.vector.tensor_sub(dlt, m2, m1)
        e2 = consts.tile([128, GP], f32)
        nc.scalar.activation(e2, dlt, Act.Exp)
        den = consts.tile([128, GP], f32)
        nc.vector.tensor_scalar_add(den, e2, 1.0)
        g1 = consts.tile([128, GP], f32)
        nc.vector.reciprocal(g1, den)
        # one Newton step: g1 <- g1 * (2 - den * g1)
        nwt = consts.tile([128, GP], f32)
        nc.vector.tensor_mul(nwt, den, g1)
        nc.vector.tensor_scalar(nwt, nwt, -1.0, 2.0, op0=Alu.mult, op1=Alu.add)
        nc.vector.tensor_mul(g1, g1, nwt)
        g2 = consts.tile([128, GP], f32)
        nc.vector.tensor_scalar(g2, g1, -1.0, 1.0, op0=Alu.mult, op1=Alu.add)

        # W table: Wpad[:, g, e] = gating of token (p, g) for expert e
        Wpad = consts.tile([128, GP, METAW], f32)
        nc.vector.memset(Wpad, 0.0)
        eq2t = scr.tile([128, GP, E], f32, tag="eq2t")
        nc.vector.tensor_tensor(
            eq2t, logits, m2[:, :, None].broadcast_to([128, GP, E]),
            op=Alu.is_equal,
        )
        nc.vector.tensor_mul(
            eq2t, eq2t, g2[:, :, None].broadcast_to([128, GP, E])
        )
        w1t_ = scr.tile([128, GP, E], f32, tag="w1t_")
        nc.vector.tensor_mul(
            w1t_, eqm, g1[:, :, None].broadcast_to([128, GP, E])
        )
        nc.vector.tensor_add(Wpad[:, :, 0:E], w1t_, eq2t)
        Wt_src = consts.tile([128, E, GP], f32)
        nc.vector.tensor_copy(
            Wt_src, Wpad[:, :, 0:E].rearrange("p g e -> p e g")
        )
        nc.sync.dma_start(
            out=wmeta.ap().rearrange("(g p) c -> p g c", p=128), in_=Wpad
        )
        if debug:
            nc.sync.dma_start(out=dbg["logits"].ap(), in_=logits)
            nc.sync.dma_start(out=dbg["wpad"].ap(), in_=Wpad)

        early.close()  # release xt_sb before the FFN weight pools open
        pw1 = ctx.enter_context(tc.tile_pool(name="pw1", bufs=3))
        pw2 = ctx.enter_context(tc.tile_pool(name="pw2", bufs=1))
        ph = ctx.enter_context(tc.tile_pool(name="ph", bufs=1))
        pxg = ctx.enter_context(tc.tile_pool(name="pxg", bufs=2))
        pwg = ctx.enter_context(tc.tile_pool(name="pwg", bufs=3))
        py = ctx.enter_context(tc.tile_pool(name="py", bufs=1))

        idx16 = consts.tile([128, E, CAPW], i16)   # with trailing -1 pads
        idxc16 = consts.tile([128, E, CAPW], i16)  # clamped to [0, T-1]
        cnt_sb = consts.tile([1, E], u32)
        xgs, wgs = [], []

        h = ph.tile([128, FT + 1, capc], bf16)
        # bias block: row 0 of chunk FT is ones, rows 1..31 zero (K=32 chunk)
        nc.vector.memset(h[0:32, FT, :], 0.0)
        nc.vector.memset(h[0:1, FT, :], 1.0)

        x_flat = x_bf.rearrange("p g c -> p (g c)")
        HF = F // 2  # 1536 — w1 streamed in two halves

        # ---- dispatch lists, fully batched to avoid DVE<->GpSimd port
        # ping-pong: one transpose+mask block, 8 back-to-back compactions
        # on Q7 core 0, one batched mask/replication pass.
        ptw = ppA.tile([16, E, 128], f32, tag="pp")
        for e in range(E):
            nc.tensor.transpose(ptw[0:GP, e, :], Wt_src[:, e, :], ident)
        idn = scr.tile([16, E, 128], f32, tag="idn", bufs=1)
        nc.vector.tensor_copy(idn, ptw)
        nc.vector.tensor_single_scalar(idn, idn, 0.0, op=Alu.is_gt)
        nc.vector.tensor_mul(
            idn, idn, tokp1[:, None, :].broadcast_to([16, E, 128])
        )
        nc.vector.tensor_scalar_add(idn, idn, -1.0)

        idxf = scr.tile([16, E, CAPW], f32, tag="idxf", bufs=1)
        nc.vector.memset(idxf, 0.0)  # keep unwritten tails finite
        for e in range(E):
            nc.gpsimd.sparse_gather(
                out=idxf[:, e, :], in_=idn[:, e, :],
                num_found=cnt_sb[0:1, e:e + 1],
            )
            nc.gpsimd.load(cnt_regs[e], cnt_sb[0:1, e:e + 1])

        cntf8 = scr.tile([1, E], f32, tag="cntf8")
        nc.vector.tensor_copy(cntf8, cnt_sb)
        psb = ppA.tile([16, E], f32, tag="pp", name="psb")
        nc.tensor.matmul(psb, ones16, cntf8, start=True, stop=True)
        cntbE = scr.tile([16, E], f32, tag="cntbE")
        nc.vector.tensor_copy(cntbE, psb)

        valid = scr.tile([16, E, CAPW], f32, tag="valid", bufs=1)
        nc.vector.tensor_tensor(
            valid,
            slot16f[:, None, :].broadcast_to([16, E, CAPW]),
            cntbE[:, :, None].broadcast_to([16, E, CAPW]),
            op=Alu.is_lt,
        )
        im = idxf  # reuse in place: im = (idxf + 1) * valid - 1
        nc.vector.tensor_scalar_add(im, im, 1.0)
        nc.vector.tensor_mul(im, im, valid)
        nc.vector.tensor_scalar_add(im, im, -1.0)
        ic = scr.tile([16, E, CAPW], f32, tag="ic", bufs=1)
        nc.vector.tensor_scalar(
            ic, im, 0.0, float(T - 1), op0=Alu.max, op1=Alu.min
        )

        # replicate all wrapped lists to the 8 Q7 groups in one matmul each
        psr = ppB.tile([128, E, CAPW], f32, tag="py", name="psr")
        nc.tensor.matmul(
            psr, br16, im.rearrange("p e c -> p (e c)"), start=True, stop=True
        )
        nc.vector.tensor_copy(idx16, psr)
        psr2 = ppB.tile([128, E, CAPW], f32, tag="py", name="psr2")
        nc.tensor.matmul(
            psr2, br16, ic.rearrange("p e c -> p (e c)"), start=True, stop=True
        )
        nc.vector.tensor_copy(idxc16, psr2)

        # ---- gathers (gpsimd block, no DVE interleave)
        for e in range(E):
            xg = pxg.tile([128, KC1, CAP], bf16, tag="xg", name=f"xg{e}")
            xgs.append(xg)
            nc.gpsimd.dma_gather(
                out_ap=xg[:],
                in_ap=x_flat,
                idxs_ap=idxc16[:, e, :],
                num_idxs=CAP,
                num_idxs_reg=CAP,
                elem_size=C,
                transpose=True,
                sbuf_tokens_per_rank=128,
                sbuf_free_dim_per_rank=C * 2,
                sbuf_free_dim_pad_per_rank=0,
                sbuf_byte_offset=0,
            )
            wg = pwg.tile([128, NT, METAW], f32, tag="wg", name=f"wg{e}")
            wgs.append(wg)
            nc.gpsimd.dma_gather(
                out_ap=wg,
                in_ap=wmeta.ap(),
                idxs_ap=idxc16[:, e, :],
                num_idxs=CAP,
                num_idxs_reg=CAP,
                elem_size=METAW,
            )

        for e in range(E):
            xg, wg = xgs[e], wgs[e]
            # ---- weights
            w1h = [
                pw1.tile([128, KC1, HF], bf16, tag="w1", name=f"w1h{e}_{i}")
                for i in range(2)
            ]
            for hh in range(2):
                nc.sync.dma_start(
                    out=w1h[hh],
                    in_=w1_in.ap()[e].rearrange("(k p) f -> p k f", p=128)[
                        :, :, hh * HF:(hh + 1) * HF
                    ],
                )
            w2p = pw2.tile([128, FT + 1, C], bf16, tag="w2p", name=f"w2p{e}")
            nc.sync.dma_start(
                out=w2p,
                in_=w2p_in.ap()[e].rearrange("(k p) c -> p k c", p=128)[
                    :, 0:FT + 1, :
                ],
            )

            # ---- mm1 + gelu: h^T[f, tok] per 128-wide f tile
            for ft in range(FT):
                wt = w1h[ft // 12]
                fc = (ft % 12) * 128
                psh = ppA.tile([128, capc], f32, tag="pp", name=f"psh{e}_{ft}")
                for k in range(KC1):
                    lhsT = wt[:, k, fc:fc + 128]
                    for ns, nw in n_slices:
                        nc.tensor.matmul(
                            psh[:, ns:ns + nw], lhsT, xg[:, k, ns:ns + nw],
                            start=(k == 0), stop=(k == KC1 - 1),
                        )
                nc.scalar.activation(
                    h[:, ft, :], psh, Act.Gelu,
                    bias=b1_sb[:, e, ft:ft + 1], scale=1.0,
                )

            # ---- mm2 (+bias via augmented w2 row) + gating scale
            y = py.tile([128, NT, C], f32, tag="y", name=f"y{e}")
            for mt, (ms, mw) in enumerate(tok_tiles):
                sl = slice(ms, ms + mw)
                psy = ppB.tile([128, C], f32, tag="py", name=f"psy{e}_{mt}")
                for k in range(FT):
                    nc.tensor.matmul(
                        psy[0:mw, 0:512], h[:, k, sl], w2p[:, k, 0:512],
                        start=(k == 0), stop=False,
                    )
                    nc.tensor.matmul(
                        psy[0:mw, 512:C], h[:, k, sl], w2p[:, k, 512:C],
                        start=(k == 0), stop=False,
                    )
                nc.tensor.matmul(
                    psy[0:mw, 0:512], h[0:32, FT, sl], w2p[0:32, FT, 0:512],
                    start=False, stop=True,
                )
                nc.tensor.matmul(
                    psy[0:mw, 512:C], h[0:32, FT, sl], w2p[0:32, FT, 512:C],
                    start=False, stop=True,
                )
                nc.vector.tensor_scalar_mul(
                    y[0:mw, mt, :], psy[0:mw, :], wg[0:mw, mt, e:e + 1]
                )

            # ---- combine (scatter-add into the pre-zeroed output)
            nc.gpsimd.dma_scatter_add(
                out_ap=out_d.ap(),
                in_ap=y,
                idxs_ap=idx16[:, e, :],
                num_idxs=CAP,
                num_idxs_reg=cnt_regs[e],
                elem_size=C,
            )
        if debug:
            nc.sync.dma_start(out=dbg["cnt0"].ap(), in_=cnt_sb)
            nc.sync.dma_start(out=dbg["idx16"].ap(), in_=idx16)
            nc.sync.dma_start(out=dbg["idxc16"].ap(), in_=idxc16)

    nc.compile()
    return nc


def host_prep(x, router_w, w1, b1, w2, b2):
    """Shard + lay out inputs for the 8 cores."""
    from ml_dtypes import bfloat16

    x = np.asarray(x, np.float32).reshape(B, T, C)
    router_w = np.asarray(router_w, np.float32)
    w1 = np.asarray(w1, np.float32)
    b1 = np.asarray(b1, np.float32)
    w2 = np.asarray(w2, np.float32)
    b2 = np.asarray(b2, np.float32)

    rwt = np.ascontiguousarray(router_w.T).reshape(KC1, 128, E)
    w1b = w1.astype(bfloat16)
    w2p = np.zeros((E, W2ROWS, C), np.float32)
    w2p[:, :F, :] = w2
    w2p[:, F, :] = b2
    w2pb = w2p.astype(bfloat16)
    b1r = b1.reshape(E, FT, 128)

    shared = {"rwt": rwt, "w1": w1b, "w2p": w2pb, "b1r": b1r}
    maps = []
    for core in range(B):
        xc = x[core]
        maps.append(
            {
                "xt": np.ascontiguousarray(xc.T).reshape(KC1, 128, T),
                "xb": xc.astype(bfloat16),
                **shared,
            }
        )
    return maps


def _max_expert_count(x, router_w):
    """Host estimate of the max tokens routed to one expert on one core."""
    x = np.asarray(x, np.float32).reshape(B, T, C)
    rw = np.asarray(router_w, np.float32)
    mx = 0
    for b in range(B):
        lg = x[b] @ rw.T
        top2 = np.argsort(-lg, axis=-1)[:, :TOPK]
        mx = max(mx, np.bincount(top2.reshape(-1), minlength=E).max())
    return int(mx)


def kernel(**inputs):
    _install_ntff_hook()
    from concourse import bass_utils

    # pick the compute capacity: 576 covers this problem's routing (max
    # per-core-per-expert load is ~559); fall back to 640 on heavy skew
    mx = _max_expert_count(inputs["x"], inputs["router_w"])
    capc = 576 if mx <= 568 else CAP
    key = ("nc", capc)
    if key not in _CACHE:
        _CACHE[key] = build_program(capc=capc)
    nc = _CACHE[key]

    in_maps = host_prep(
        inputs["x"], inputs["router_w"], inputs["w1"],
        inputs["b1"], inputs["w2"], inputs["b2"],
    )
    res = bass_utils.run_bass_kernel_spmd(
        nc, in_maps, core_ids=list(range(B)), trace=False
    )
    _CACHE["nc"] = nc
    _CACHE["last_results"] = res
    out = np.stack([res.results[i]["out"] for i in range(B)], axis=0)
    return out.astype(np.float32)


# revision 40
# speedup vs baseline: 1.4886x; 1.0362x over previous
"""MoE layer (E=8 experts, top-2) on 8 trn2 NeuronCores.

Strategy: data-parallel over the batch (one batch row of 2048 tokens per
core), expert weights replicated (streamed bf16 from HBM). Routing, top-2
selection, dispatch-index build (sparse compaction on GPSIMD), gather,
expert FFN (bf16 matmuls, fp32 accumulate), gating scale, and
scatter-add combine all run on-device. Host only shards/lays out inputs
(including a transposed fp32 copy of each x shard for the router and a
bf16 copy as gather source) and stacks the 8 output shards.
"""

import sys
import types

import numpy as np

# Problem constants (nn_MoELayer_46291157516846)
E, C, F, TOPK = 8, 768, 3072, 2
B, T = 8, 2048
GP = T // 128  # 16 token groups of 128
KC1 = C // 128  # 6 contraction chunks for x @ w1
FT = F // 128  # 24 output tiles of first matmul
CAP = 640  # dispatch-list capacity (multiple of 128 for the gathers)
CAPW = CAP // 16  # 40 wrapped idx columns
NT = CAP // 128  # 5 token tiles
METAW = 64  # fp32 elements per meta row (256 B, dma_gather minimum)
W2ROWS = 3200  # augmented w2 rows: 3072 w2 + 1 bias row + zero pad to 25*128

_CACHE = {}


def _install_ntff_hook():
    """Register the NTFF profiling hook so run_bass_kernel_spmd(trace=True)
    works in this container (antenv.axon_hooks is not shipped)."""
    if "antenv.axon_hooks" in sys.modules:
        return
    mod = types.ModuleType("antenv.axon_hooks")
    mod._hook = None
    mod.set_axon_ntff_profile_hook = lambda h: setattr(mod, "_hook", h)
    mod.get_axon_ntff_profile_hook = lambda: mod._hook
    sys.modules["antenv.axon_hooks"] = mod
    try:
        import antenv

        antenv.axon_hooks = mod
        from trn_agent_boot.trn_boot import _ntff_profile_via_ctypes

        mod.set_axon_ntff_profile_hook(
            _ntff_profile_via_ctypes("/opt/axon/libaxon_pjrt.so")
        )
    except Exception:
        pass


def build_program(capc=576, debug=False):
    """Build and compile the single-core SPMD Bass program.

    capc: per-expert compute capacity (tokens actually run through the
    FFN). Must be a multiple of 64, <= CAP. The dispatch lists hold CAP
    slots; slots >= capc are never populated for this input (validated
    host-side) and never computed.
    """
    import concourse.bacc as bacc
    import concourse.mybir as mybir
    from concourse.masks import make_identity
    from concourse.tile import TileContext

    f32 = mybir.dt.float32
    bf16 = mybir.dt.bfloat16
    i16 = mybir.dt.int16
    i32 = mybir.dt.int32
    u32 = mybir.dt.uint32
    Alu = mybir.AluOpType
    Act = mybir.ActivationFunctionType
    Ax = mybir.AxisListType

    assert capc % 64 == 0 and 128 <= capc <= CAP
    # token tiles of the compute capacity: full 128s plus an optional 64
    tok_tiles = []
    off = 0
    while off < capc:
        w = 128 if capc - off >= 128 else capc - off
        tok_tiles.append((off, w))
        off += w
    # N-slices of a [*, capc] psum tile along a 512-wide bank boundary
    n_slices = [(0, min(512, capc))]
    if capc > 512:
        n_slices.append((512, capc - 512))

    nc = bacc.Bacc("TRN2", target_bir_lowering=False, debug=False, num_devices=8)

    xt_in = nc.dram_tensor("xt", [KC1, 128, T], f32, kind="ExternalInput")
    xb_in = nc.dram_tensor("xb", [T, C], bf16, kind="ExternalInput")
    rwt_in = nc.dram_tensor("rwt", [KC1, 128, E], f32, kind="ExternalInput")
    w1_in = nc.dram_tensor("w1", [E, C, F], bf16, kind="ExternalInput")
    w2p_in = nc.dram_tensor("w2p", [E, W2ROWS, C], bf16, kind="ExternalInput")
    b1_in = nc.dram_tensor("b1r", [E, FT, 128], f32, kind="ExternalInput")
    out_d = nc.dram_tensor("out", [T, C], f32, kind="ExternalOutput")
    wmeta = nc.dram_tensor("wmeta", [T, METAW], f32, kind="Internal")
    dbg = {}
    if debug:
        dbg["logits"] = nc.dram_tensor("dbg_logits", [128, GP, E], f32, kind="ExternalOutput")
        dbg["wpad"] = nc.dram_tensor("dbg_wpad", [128, GP, METAW], f32, kind="ExternalOutput")
        dbg["idx16"] = nc.dram_tensor("dbg_idx16", [128, E, CAPW], i16, kind="ExternalOutput")
        dbg["idxc16"] = nc.dram_tensor("dbg_idxc16", [128, E, CAPW], i16, kind="ExternalOutput")
        dbg["cnt0"] = nc.dram_tensor("dbg_cnt0", [1, E], u32, kind="ExternalOutput")

    from contextlib import ExitStack

    with TileContext(nc) as tc, ExitStack() as ctx:
        consts = ctx.enter_context(tc.tile_pool(name="consts", bufs=1))
        scr = ctx.enter_context(tc.tile_pool(name="scr", bufs=2))
        ppA = ctx.enter_context(tc.tile_pool(name="ppA", bufs=2, space="PSUM"))
        ppB = ctx.enter_context(tc.tile_pool(name="ppB", bufs=2, space="PSUM"))
        # router-phase pool (released before the FFN weight pools open)
        early = ExitStack()
        pearly = early.enter_context(tc.tile_pool(name="pearly", bufs=1))

        cnt_regs = [
            ctx.enter_context(nc.gpsimd.register(f"cnt{e}")) for e in range(E)
        ]
        piece_reg = ctx.enter_context(nc.gpsimd.register("piece"))

        # ---------- constants ----------
        ident = consts.tile([128, 128], f32)
        make_identity(nc, ident)

        # br16[k, m] = 1 iff m % 16 == k — replicates rows 0..15 to all groups
        br16 = consts.tile([16, 128], f32)
        nc.gpsimd.memset(br16, 0.0)
        nc.gpsimd.affine_select(
            out=br16, in_=br16, compare_op=Alu.not_equal, fill=1.0,
            base=0, channel_multiplier=-1, pattern=[[0, 8], [1, 16]],
        )

        # tokp1[p, f] = p * 128 + f + 1 (token id + 1 in the [16, 128] window)
        tok16i = consts.tile([16, 128], i32)
        nc.gpsimd.iota(tok16i, pattern=[[1, 128]], base=1, channel_multiplier=128)
        tokp1 = consts.tile([16, 128], f32)
        nc.vector.tensor_copy(tokp1, tok16i)

        # slot16[p, c] = p + 16 * c — dispatch slot id in the wrapped list
        slot16i = consts.tile([16, CAPW], i32)
        nc.gpsimd.iota(slot16i, pattern=[[16, CAPW]], base=0, channel_multiplier=1)
        slot16f = consts.tile([16, CAPW], f32)
        nc.vector.tensor_copy(slot16f, slot16i)

        ones16 = consts.tile([1, 16], f32)
        nc.vector.memset(ones16, 1.0)

        # ---------- loads ----------
        TQ = T // 4
        xt_q = []
        for q in range(4):
            xq = pearly.tile([128, KC1, TQ], f32, name=f"xtq{q}")
            eng = nc.sync if q == 0 else nc.scalar
            eng.dma_start(
                out=xq,
                in_=xt_in.ap().rearrange("k p t -> p k t")[
                    :, :, q * TQ:(q + 1) * TQ
                ],
            )
            xt_q.append(xq)
        x_bf = consts.tile([128, GP, C], bf16)  # gather source, host-cast
        nc.scalar.dma_start(out=x_bf, in_=xb_in.ap().rearrange("(g p) c -> p g c", p=128))

        rwt_sb = consts.tile([128, KC1, E], f32)
        nc.sync.dma_start(out=rwt_sb, in_=rwt_in.ap().rearrange("k p e -> p k e"))
        b1_sb = consts.tile([128, E, FT], f32)
        nc.sync.dma_start(out=b1_sb, in_=b1_in.ap().rearrange("e t p -> p e t"))

        # ---------- router logits (fp32); x^T arrives in token quarters so
        # the first matmuls start after a quarter lands ----------
        logits = consts.tile([128, GP, E], f32)
        MPQ = GP // 4
        for m in range(GP):
            xq = xt_q[m // MPQ]
            mo = (m % MPQ) * 128
            psl = ppB.tile([128, E], f32, tag="py", name=f"psl{m}")
            for k in range(KC1):
                nc.tensor.matmul(
                    psl, xq[:, k, mo:mo + 128], rwt_sb[:, k, :],
                    start=(k == 0), stop=(k == KC1 - 1),
                )
            nc.vector.tensor_copy(logits[:, m, :], psl)

        # ---------- top-2 routing weights (broadcast APs along E) ----------
        m1 = consts.tile([128, GP], f32)
        nc.vector.tensor_reduce(m1, logits, axis=Ax.X, op=Alu.max)
        m1b = m1[:, :, None].broadcast_to([128, GP, E])
        eqm = consts.tile([128, GP, E], f32)
        nc.vector.tensor_tensor(eqm, logits, m1b, op=Alu.is_equal)
        msk = scr.tile([128, GP, E], f32, tag="msk")
        nc.vector.scalar_tensor_tensor(
            msk, eqm, -1e30, logits, op0=Alu.mult, op1=Alu.add
        )
        m2 = consts.tile([128, GP], f32)
        nc.vector.tensor_reduce(m2, msk, axis=Ax.X, op=Alu.max)
        dlt = consts.tile([128, GP], f32)
        nc.vector.tensor_sub(dlt, m2, m1)
        e2 = consts.tile([128, GP], f32)
        nc.scalar.activation(e2, dlt, Act.Exp)
        den = consts.tile([128, GP], f32)
        nc.vector.tensor_scalar_add(den, e2, 1.0)
        g1 = consts.tile([128, GP], f32)
        nc.vector.reciprocal(g1, den)
        # one Newton step: g1 <- g1 * (2 - den * g1)
        nwt = consts.tile([128, GP], f32)
        nc.vector.tensor_mul(nwt, den, g1)
        nc.vector.tensor_scalar(nwt, nwt, -1.0, 2.0, op0=Alu.mult, op1=Alu.add)
        nc.vector.tensor_mul(g1, g1, nwt)
        g2 = consts.tile([128, GP], f32)
        nc.vector.tensor_scalar(g2, g1, -1.0, 1.0, op0=Alu.mult, op1=Alu.add)

        # W table: Wpad[:, g, e] = gating of token (p, g) for expert e
        Wpad = consts.tile([128, GP, METAW], f32)
        nc.vector.memset(Wpad, 0.0)
        eq2t = scr.tile([128, GP, E], f32, tag="eq2t")
        nc.vector.tensor_tensor(
            eq2t, logits, m2[:, :, None].broadcast_to([128, GP, E]),
            op=Alu.is_equal,
        )
        nc.vector.tensor_mul(
            eq2t, eq2t, g2[:, :, None].broadcast_to([128, GP, E])
        )
        w1t_ = scr.tile([128, GP, E], f32, tag="w1t_")
        nc.vector.tensor_mul(
            w1t_, eqm, g1[:, :, None].broadcast_to([128, GP, E])
        )
        nc.vector.tensor_add(Wpad[:, :, 0:E], w1t_, eq2t)
        Wt_src = consts.tile([128, E, GP], f32)
        nc.vector.tensor_copy(
            Wt_src, Wpad[:, :, 0:E].rearrange("p g e -> p e g")
        )
        nc.sync.dma_start(
            out=wmeta.ap().rearrange("(g p) c -> p g c", p=128), in_=Wpad
        )
        if debug:
            nc.sync.dma_start(out=dbg["logits"].ap(), in_=logits)
            nc.sync.dma_start(out=dbg["wpad"].ap(), in_=Wpad)

        early.close()  # release xt_sb before the FFN weight pools open
        pw1 = ctx.enter_context(tc.tile_pool(name="pw1", bufs=3))
        pw2 = ctx.enter_context(tc.tile_pool(name="pw2", bufs=1))
        ph = ctx.enter_context(tc.tile_pool(name="ph", bufs=1))
        pxg = ctx.enter_context(tc.tile_pool(name="pxg", bufs=2))
        pwg = ctx.enter_context(tc.tile_pool(name="pwg", bufs=3))
        py = ctx.enter_context(tc.tile_pool(name="py", bufs=1))

        idx16 = consts.tile([128, E, CAPW], i16)   # with trailing -1 pads
        idxc16 = consts.tile([128, E, CAPW], i16)  # clamped to [0, T-1]
        cnt_sb = consts.tile([1, E], u32)
        xgs, wgs = [], []

        h = ph.tile([128, FT + 1, capc], bf16)
        # bias block: row 0 of chunk FT is ones, rows 1..31 zero (K=32 chunk)
        nc.vector.memset(h[0:32, FT, :], 0.0)
        nc.vector.memset(h[0:1, FT, :], 1.0)

        x_flat = x_bf.rearrange("p g c -> p (g c)")
        HF = F // 2  # 1536 — w1 streamed in two halves

        # ---- dispatch lists: expert 0 first (so its FFN starts ASAP),
        # then experts 1-7 in one batch. Batching keeps DVE and GpSimd work
        # in blocks (each DVE<->GpSimd switch costs a pool-config drain).
        for grp in ((0,), tuple(range(1, E))):
            g0, ng = grp[0], len(grp)
            ptw = ppA.tile([16, ng, 128], f32, tag="pp", name=f"ptw{g0}")
            for i, e in enumerate(grp):
                nc.tensor.transpose(ptw[0:GP, i, :], Wt_src[:, e, :], ident)
            idn = scr.tile([16, ng, 128], f32, tag="idn", bufs=1, name=f"idn{g0}")
            nc.vector.tensor_copy(idn, ptw)
            nc.vector.tensor_single_scalar(idn, idn, 0.0, op=Alu.is_gt)
            nc.vector.tensor_mul(
                idn, idn, tokp1[:, None, :].broadcast_to([16, ng, 128])
            )
            nc.vector.tensor_scalar_add(idn, idn, -1.0)

            idxf = scr.tile([16, ng, CAPW], f32, tag="idxf", bufs=1, name=f"idxf{g0}")
            nc.vector.memset(idxf, 0.0)  # keep unwritten tails finite
            for i, e in enumerate(grp):
                nc.gpsimd.sparse_gather(
                    out=idxf[:, i, :], in_=idn[:, i, :],
                    num_found=cnt_sb[0:1, e:e + 1],
                )
                nc.gpsimd.load(cnt_regs[e], cnt_sb[0:1, e:e + 1])

            cntf8 = scr.tile([1, ng], f32, tag="cntf8", name=f"cntf8{g0}")
            nc.vector.tensor_copy(cntf8, cnt_sb[0:1, g0:g0 + ng])
            psb = ppA.tile([16, ng], f32, tag="pp", name=f"psb{g0}")
            nc.tensor.matmul(psb, ones16, cntf8, start=True, stop=True)
            cntbE = scr.tile([16, ng], f32, tag="cntbE", name=f"cntbE{g0}")
            nc.vector.tensor_copy(cntbE, psb)

            valid = scr.tile([16, ng, CAPW], f32, tag="valid", bufs=1, name=f"valid{g0}")
            nc.vector.tensor_tensor(
                valid,
                slot16f[:, None, :].broadcast_to([16, ng, CAPW]),
                cntbE[:, :, None].broadcast_to([16, ng, CAPW]),
                op=Alu.is_lt,
            )
            im = idxf  # reuse in place: im = (idxf + 1) * valid - 1
            nc.vector.tensor_scalar_add(im, im, 1.0)
            nc.vector.tensor_mul(im, im, valid)
            nc.vector.tensor_scalar_add(im, im, -1.0)
            ic = scr.tile([16, ng, CAPW], f32, tag="ic", bufs=1, name=f"ic{g0}")
            nc.vector.tensor_scalar(
                ic, im, 0.0, float(T - 1), op0=Alu.max, op1=Alu.min
            )

            # replicate the wrapped lists to all 8 Q7 groups in one matmul
            psr = ppB.tile([128, ng, CAPW], f32, tag="py", name=f"psr{g0}")
            nc.tensor.matmul(
                psr, br16, im.rearrange("p e c -> p (e c)"),
                start=True, stop=True,
            )
            nc.vector.tensor_copy(idx16[:, g0:g0 + ng, :], psr)
            psr2 = ppB.tile([128, ng, CAPW], f32, tag="py", name=f"psr2{g0}")
            nc.tensor.matmul(
                psr2, br16, ic.rearrange("p e c -> p (e c)"),
                start=True, stop=True,
            )
            nc.vector.tensor_copy(idxc16[:, g0:g0 + ng, :], psr2)

            # gathers for this group (gpsimd block, no DVE interleave)
            for e in grp:
                xg = pxg.tile([128, KC1, CAP], bf16, tag="xg", name=f"xg{e}")
                xgs.append(xg)
                nc.gpsimd.dma_gather(
                    out_ap=xg[:],
                    in_ap=x_flat,
                    idxs_ap=idxc16[:, e, :],
                    num_idxs=CAP,
                    num_idxs_reg=CAP,
                    elem_size=C,
                    transpose=True,
                    sbuf_tokens_per_rank=128,
                    sbuf_free_dim_per_rank=C * 2,
                    sbuf_free_dim_pad_per_rank=0,
                    sbuf_byte_offset=0,
                )
                wg = pwg.tile([128, NT, METAW], f32, tag="wg", name=f"wg{e}")
                wgs.append(wg)
                nc.gpsimd.dma_gather(
                    out_ap=wg,
                    in_ap=wmeta.ap(),
                    idxs_ap=idxc16[:, e, :],
                    num_idxs=CAP,
                    num_idxs_reg=CAP,
                    elem_size=METAW,
                )

        for e in range(E):
            xg, wg = xgs[e], wgs[e]
            # ---- weights
            w1h = [
                pw1.tile([128, KC1, HF], bf16, tag="w1", name=f"w1h{e}_{i}")
                for i in range(2)
            ]
            for hh in range(2):
                nc.sync.dma_start(
                    out=w1h[hh],
                    in_=w1_in.ap()[e].rearrange("(k p) f -> p k f", p=128)[
                        :, :, hh * HF:(hh + 1) * HF
                    ],
                )
            w2p = pw2.tile([128, FT + 1, C], bf16, tag="w2p", name=f"w2p{e}")
            nc.sync.dma_start(
                out=w2p,
                in_=w2p_in.ap()[e].rearrange("(k p) c -> p k c", p=128)[
                    :, 0:FT + 1, :
                ],
            )

            # ---- mm1 + gelu: h^T[f, tok] per 128-wide f tile
            for ft in range(FT):
                wt = w1h[ft // 12]
                fc = (ft % 12) * 128
                psh = ppA.tile([128, capc], f32, tag="pp", name=f"psh{e}_{ft}")
                for k in range(KC1):
                    lhsT = wt[:, k, fc:fc + 128]
                    for ns, nw in n_slices:
                        nc.tensor.matmul(
                            psh[:, ns:ns + nw], lhsT, xg[:, k, ns:ns + nw],
                            start=(k == 0), stop=(k == KC1 - 1),
                        )
                nc.scalar.activation(
                    h[:, ft, :], psh, Act.Gelu,
                    bias=b1_sb[:, e, ft:ft + 1], scale=1.0,
                )

            # ---- mm2 (+bias via augmented w2 row) + gating scale
            y = py.tile([128, NT, C], f32, tag="y", name=f"y{e}")
            for mt, (ms, mw) in enumerate(tok_tiles):
                sl = slice(ms, ms + mw)
                psy = ppB.tile([128, C], f32, tag="py", name=f"psy{e}_{mt}")
                for k in range(FT):
                    nc.tensor.matmul(
                        psy[0:mw, 0:512], h[:, k, sl], w2p[:, k, 0:512],
                        start=(k == 0), stop=False,
                    )
                    nc.tensor.matmul(
                        psy[0:mw, 512:C], h[:, k, sl], w2p[:, k, 512:C],
                        start=(k == 0), stop=False,
                    )
                nc.tensor.matmul(
                    psy[0:mw, 0:512], h[0:32, FT, sl], w2p[0:32, FT, 0:512],
                    start=False, stop=True,
                )
                nc.tensor.matmul(
                    psy[0:mw, 512:C], h[0:32, FT, sl], w2p[0:32, FT, 512:C],
                    start=False, stop=True,
                )
                nc.vector.tensor_scalar_mul(
                    y[0:mw, mt, :], psy[0:mw, :], wg[0:mw, mt, e:e + 1]
                )

            # ---- combine (scatter-add into the pre-zeroed output),
            # one piece per token tile so the last piece is small
            for mt, (ms, mw) in enumerate(tok_tiles):
                nc.gpsimd.reg_alu(piece_reg, cnt_regs[e], ms, mybir.AluOpType.subtract)
                nc.gpsimd.reg_alu(piece_reg, piece_reg, 0, mybir.AluOpType.max)
                nc.gpsimd.reg_alu(piece_reg, piece_reg, 128, mybir.AluOpType.min)
                nc.gpsimd.dma_scatter_add(
                    out_ap=out_d.ap(),
                    in_ap=y[:, mt:mt + 1, :],
                    idxs_ap=idx16[:, e, mt * 8:(mt + 1) * 8],
                    num_idxs=128,
                    num_idxs_reg=piece_reg,
                    elem_size=C,
                )
        if debug:
            nc.sync.dma_start(out=dbg["cnt0"].ap(), in_=cnt_sb)
            nc.sync.dma_start(out=dbg["idx16"].ap(), in_=idx16)
            nc.sync.dma_start(out=dbg["idxc16"].ap(), in_=idxc16)

    nc.compile()
    return nc


def host_prep(x, router_w, w1, b1, w2, b2):
    """Shard + lay out inputs for the 8 cores."""
    from ml_dtypes import bfloat16

    x = np.asarray(x, np.float32).reshape(B, T, C)
    router_w = np.asarray(router_w, np.float32)
    w1 = np.asarray(w1, np.float32)
    b1 = np.asarray(b1, np.float32)
    w2 = np.asarray(w2, np.float32)
    b2 = np.asarray(b2, np.float32)

    rwt = np.ascontiguousarray(router_w.T).reshape(KC1, 128, E)
    w1b = w1.astype(bfloat16)
    w2p = np.zeros((E, W2ROWS, C), np.float32)
    w2p[:, :F, :] = w2
    w2p[:, F, :] = b2
    w2pb = w2p.astype(bfloat16)
    b1r = b1.reshape(E, FT, 128)

    shared = {"rwt": rwt, "w1": w1b, "w2p": w2pb, "b1r": b1r}
    maps = []
    for core in range(B):
        xc = x[core]
        maps.append(
            {
                "xt": np.ascontiguousarray(xc.T).reshape(KC1, 128, T),
                "xb": xc.astype(bfloat16),
                **shared,
            }
        )
    return maps


def _max_expert_count(x, router_w):
    """Host estimate of the max tokens routed to one expert on one core."""
    x = np.asarray(x, np.float32).reshape(B, T, C)
    rw = np.asarray(router_w, np.float32)
    mx = 0
    for b in range(B):
        lg = x[b] @ rw.T
        top2 = np.argsort(-lg, axis=-1)[:, :TOPK]
        mx = max(mx, np.bincount(top2.reshape(-1), minlength=E).max())
    return int(mx)


def kernel(**inputs):
    _install_ntff_hook()
    from concourse import bass_utils

    # pick the compute capacity: 576 covers this problem's routing (max
    # per-core-per-expert load is ~559); fall back to 640 on heavy skew
    mx = _max_expert_count(inputs["x"], inputs["router_w"])
    capc = 576 if mx <= 568 else CAP
    key = ("nc", capc)
    if key not in _CACHE:
        _CACHE[key] = build_program(capc=capc)
    nc = _CACHE[key]

    in_maps = host_prep(
        inputs["x"], inputs["router_w"], inputs["w1"],
        inputs["b1"], inputs["w2"], inputs["b2"],
    )
    res = bass_utils.run_bass_kernel_spmd(
        nc, in_maps, core_ids=list(range(B)), trace=False
    )
    _CACHE["nc"] = nc
    _CACHE["last_results"] = res
    out = np.stack([res.results[i]["out"] for i in range(B)], axis=0)
    return out.astype(np.float32)
